# revision 14
# baseline (speedup 1.0000x reference)
"""Trainium2 Bass kernel for nn_AdaptiveRouter (MoE dual-gate routing).

8 NeuronCores, data-parallel over tokens. Each core handles 512 tokens:
  - fp32 matmuls for importance MLP + both routers (decisions are
    numerically sensitive: mask threshold and top-k tie distances)
  - top-2 selection via exact fp32 compares on logits
  - capacity positions: strict-upper-triangular prefix matmuls within a
    128-token chunk, chunk carries, and an 8-core AllGather of per-core
    per-(k,expert) counts for the global k-major cumsum offsets
  - dispatch/combine [512,8,1536] f32 shards: zero-filled with large
    DMA stores, then one 1536-float row per (token,k) scattered via
    indirect DMA (OOB row index drops capacity-overflow entries)
  - second AllGather for aux-loss partial sums; ln() via series
"""

import math
import os
import sys
import types

sys.path.insert(0, "/opt/trn_rl_repo")

import numpy as np

# ---- problem constants (hardcoded; kernel.py must be self-contained) ----
B, S, H, E, TOPK, CF = 2, 2048, 1024, 8, 2, 1.5
N = B * S                      # 4096 tokens
NCORES = 8
TPC = N // NCORES              # 512 tokens per core
CAP = int(N * CF * TOPK / E)   # 1536
P = 128
NCH = TPC // P                 # 4 token chunks per core
H2 = H // 2
BIG = 1.0e6                    # row marker for dropped entries (OOB -> skipped)
LN8 = math.log(8.0)
NTERMS = 12                    # ln(1+x) series terms

LAST_EXEC_NS = None
LAST_TRACE_DIR = None

_CACHE = {}


def _install_ntff_hook():
    """Recreate antenv.axon_hooks (absent in this image) so
    run_bass_kernel_spmd(trace=True) can profile via libaxon_pjrt."""
    import antenv

    if "antenv.axon_hooks" not in sys.modules:
        mod = types.ModuleType("antenv.axon_hooks")
        mod._hook = None

        def set_axon_ntff_profile_hook(h):
            mod._hook = h

        def get_axon_ntff_profile_hook():
            return mod._hook

        mod.set_axon_ntff_profile_hook = set_axon_ntff_profile_hook
        mod.get_axon_ntff_profile_hook = get_axon_ntff_profile_hook
        sys.modules["antenv.axon_hooks"] = mod
        antenv.axon_hooks = mod
    mod = sys.modules["antenv.axon_hooks"]
    if mod._hook is None:
        from trn_agent_boot.trn_boot import _ntff_profile_via_ctypes

        mod.set_axon_ntff_profile_hook(
            _ntff_profile_via_ctypes("/opt/axon/libaxon_pjrt.so")
        )


def _build():
    import concourse.bacc as bacc
    import concourse.mybir as mybir
    import concourse.tile as tile
    import concourse.bass as bass
    from concourse.masks import make_identity, make_upper_triangular
    from concourse.tile import add_dep_helper
    from contextlib import ExitStack

    fp32 = mybir.dt.float32
    i32 = mybir.dt.int32
    AF = mybir.ActivationFunctionType
    OP = mybir.AluOpType
    AX = mybir.AxisListType

    nc = bacc.Bacc(None, target_bir_lowering=False)

    x_d = nc.declare_dram_parameter("hidden_states", [TPC, H], fp32, isOutput=False)
    wi1_d = nc.declare_dram_parameter("wi1", [H, H2], fp32, isOutput=False)
    bi1_d = nc.declare_dram_parameter("bi1", [H2], fp32, isOutput=False)
    wi2_d = nc.declare_dram_parameter("wi2", [H2, 1], fp32, isOutput=False)
    bi2_d = nc.declare_dram_parameter("bi2", [1, 1], fp32, isOutput=False)
    wr1_d = nc.declare_dram_parameter("wr1", [H, H], fp32, isOutput=False)
    br1_d = nc.declare_dram_parameter("br1", [H], fp32, isOutput=False)
    wr2_d = nc.declare_dram_parameter("wr2", [H, E], fp32, isOutput=False)
    br2_d = nc.declare_dram_parameter("br2", [1, E], fp32, isOutput=False)
    wu1_d = nc.declare_dram_parameter("wu1", [H, H], fp32, isOutput=False)
    bu1_d = nc.declare_dram_parameter("bu1", [H], fp32, isOutput=False)
    wu2_d = nc.declare_dram_parameter("wu2", [H, E], fp32, isOutput=False)
    bu2_d = nc.declare_dram_parameter("bu2", [1, E], fp32, isOutput=False)
    cm_d = nc.declare_dram_parameter("cmask", [NCORES, 1], fp32, isOutput=False)

    disp_d = nc.declare_dram_parameter("dispatch", [TPC * E, CAP], fp32, isOutput=True)
    comb_d = nc.declare_dram_parameter("combine", [TPC * E, CAP], fp32, isOutput=True)
    prob_d = nc.declare_dram_parameter("probs", [TPC, E], fp32, isOutput=True)
    imp_d = nc.declare_dram_parameter("importance", [TPC, 1], fp32, isOutput=True)
    aux_d = nc.declare_dram_parameter("aux", [1, 1], fp32, isOutput=True)

    IOA = bass.IndirectOffsetOnAxis
    RG = [list(range(NCORES))]

    with tile.TileContext(nc) as tc, ExitStack() as ctx:
        const = ctx.enter_context(tc.tile_pool(name="const", bufs=1))
        wpool = ctx.enter_context(tc.tile_pool(name="wpool", bufs=1))
        persist = ctx.enter_context(tc.tile_pool(name="persist", bufs=1))
        xraw_p = ctx.enter_context(tc.tile_pool(name="xraw", bufs=2))
        sm = ctx.enter_context(tc.tile_pool(name="sm", bufs=3))       # small transients
        rowp = ctx.enter_context(tc.tile_pool(name="rowp", bufs=3))   # [P,CAP] rows
        l1ps = ctx.enter_context(tc.tile_pool(name="l1ps", bufs=2, space="PSUM"))
        l2ps = ctx.enter_context(tc.tile_pool(name="l2ps", bufs=2, space="PSUM"))
        ccps_p = ctx.enter_context(tc.tile_pool(name="ccps", bufs=2, space="PSUM"))
        dram = ctx.enter_context(tc.tile_pool(name="dram", bufs=1, space="DRAM"))

        def ts(out, in0, s1, op0, s2=None, op1=None, eng=None):
            e = eng if eng is not None else nc.vector
            if s2 is None:
                e.tensor_scalar(out=out, in0=in0, scalar1=s1, scalar2=None, op0=op0)
            else:
                e.tensor_scalar(out=out, in0=in0, scalar1=s1, scalar2=s2,
                                op0=op0, op1=op1)

        def tt(out, a, b, op):
            nc.vector.tensor_tensor(out=out, in0=a, in1=b, op=op)

        # ---------------- constants ----------------
        ident = const.tile([P, P], fp32)
        make_identity(nc, ident[:])
        ut = const.tile([P, P], fp32)
        make_upper_triangular(nc, ut[:], val=1.0, diag=False)  # strictly upper
        iota8 = const.tile([P, E], fp32)
        nc.gpsimd.iota(iota8[:], pattern=[[1, E]], base=0, channel_multiplier=0,
                       allow_small_or_imprecise_dtypes=True)
        iota_col8 = const.tile([P, 1], fp32)
        nc.gpsimd.iota(iota_col8[:], pattern=[[0, 1]], base=0, channel_multiplier=E,
                       allow_small_or_imprecise_dtypes=True)  # p*8
        iota_cap = const.tile([P, CAP], fp32)
        nc.gpsimd.iota(iota_cap[:], pattern=[[1, CAP]], base=0, channel_multiplier=0,
                       allow_small_or_imprecise_dtypes=True)
        ones_row = const.tile([1, P], fp32)
        nc.vector.memset(ones_row[:], 1.0)
        ones_col = const.tile([P, 1], fp32)
        nc.vector.memset(ones_col[:], 1.0)
        ones8 = const.tile([NCORES, 1], fp32)
        nc.vector.memset(ones8[:], 1.0)
        zero_sb = const.tile([P, 2048], fp32)
        nc.vector.memset(zero_sb[:], 0.0)

        # ---------------- x load + transpose (PE critical path first) -----
        xT = wpool.tile([P, 8, TPC], fp32)
        for tcch in range(NCH):
            x_raw = xraw_p.tile([P, H], fp32, tag="xraw")
            nc.sync.dma_start(out=x_raw[:], in_=x_d[tcch * P:(tcch + 1) * P, :])
            for kc in range(8):
                tps = l2ps.tile([P, P], fp32, tag="l2")
                nc.tensor.transpose(out=tps[:], in_=x_raw[:, kc * P:(kc + 1) * P],
                                    identity=ident[:])
                nc.vector.tensor_copy(out=xT[:, kc, tcch * P:(tcch + 1) * P], in_=tps[:])

        # ---------------- weight loads ----------------
        wi1_sb = wpool.tile([P, 8, H2], fp32)
        nc.sync.dma_start(out=wi1_sb[:], in_=wi1_d[:, :].rearrange("(k p) m -> p k m", p=P))
        wr1_sb = wpool.tile([P, 8, H], fp32)
        nc.sync.dma_start(out=wr1_sb[:], in_=wr1_d[:, :].rearrange("(k p) m -> p k m", p=P))
        wu1_sb = wpool.tile([P, 8, H], fp32)
        nc.sync.dma_start(out=wu1_sb[:], in_=wu1_d[:, :].rearrange("(k p) m -> p k m", p=P))
        wi2_sb = wpool.tile([P, 4, 1], fp32)
        nc.sync.dma_start(out=wi2_sb[:], in_=wi2_d[:, :].rearrange("(k p) m -> p k m", p=P))
        wr2_sb = wpool.tile([P, 8, E], fp32)
        nc.sync.dma_start(out=wr2_sb[:], in_=wr2_d[:, :].rearrange("(k p) m -> p k m", p=P))
        wu2_sb = wpool.tile([P, 8, E], fp32)
        nc.sync.dma_start(out=wu2_sb[:], in_=wu2_d[:, :].rearrange("(k p) m -> p k m", p=P))
        bi1c = wpool.tile([P, 4], fp32)
        nc.sync.dma_start(out=bi1c[:], in_=bi1_d[:].rearrange("(m p) -> p m", p=P))
        br1c = wpool.tile([P, 8], fp32)
        nc.sync.dma_start(out=br1c[:], in_=br1_d[:].rearrange("(m p) -> p m", p=P))
        bu1c = wpool.tile([P, 8], fp32)
        nc.sync.dma_start(out=bu1c[:], in_=bu1_d[:].rearrange("(m p) -> p m", p=P))
        br2row = wpool.tile([1, E], fp32)
        nc.sync.dma_start(out=br2row[:], in_=br2_d[:, :])
        bu2row = wpool.tile([1, E], fp32)
        nc.sync.dma_start(out=bu2row[:], in_=bu2_d[:, :])
        bi2row = wpool.tile([1, 1], fp32)
        nc.sync.dma_start(out=bi2row[:], in_=bi2_d[:, :])
        cmask_sb = wpool.tile([NCORES, 1], fp32)
        nc.sync.dma_start(out=cmask_sb[:], in_=cm_d[:, :])

        # ---------------- zero-fill dispatch/combine (fills DMA idle) -----
        zfill_insts = {"dispatch": [], "combine": []}
        for name, dd in (("dispatch", disp_d), ("combine", comb_d)):
            flat = dd[:, :].rearrange("r c -> (r c)")
            chunk = P * 2048
            for i in range((TPC * E * CAP) // chunk):   # 24 x 1MiB each
                ins = nc.sync.dma_start(
                    out=flat[i * chunk:(i + 1) * chunk].rearrange("(p f) -> p f", p=P),
                    in_=zero_sb[:],
                )
                zfill_insts[name].append(ins)

        # ---------------- persistent small tensors ----------------
        mask_sb = persist.tile([P, NCH], fp32)
        li_sb = persist.tile([P, NCH, E], fp32)
        logits_sb = persist.tile([P, NCH, E], fp32)
        rmax_sb = persist.tile([P, NCH], fp32)
        probs_sb = persist.tile([P, NCH, E], fp32)
        ohs = persist.tile([P, NCH, 2, E], fp32)
        idx_sb = persist.tile([P, NCH, 2], fp32)
        w_sb = persist.tile([P, NCH, 2], fp32)
        prefix_sb = persist.tile([P, NCH, 2, E], fp32)  # within-chunk excl prefix
        chcnt = persist.tile([1, 2, NCH, E], fp32)   # per (k, chunk) counts
        carr = persist.tile([1, 2, NCH, E], fp32)    # per (k, chunk) carry
        probsum_sb = persist.tile([1, E], fp32)
        impsum_sb = persist.tile([1, E], fp32)
        agin = persist.tile([1, 2 * E], fp32)        # counts AG payload
        agin2 = persist.tile([1, 2 * E], fp32)       # aux AG payload
        cnt_all = persist.tile([NCORES, 2 * E], fp32)
        aux_all = persist.tile([NCORES, 2 * E], fp32)
        off0 = persist.tile([1, E], fp32)
        off1 = persist.tile([1, E], fp32)
        tot0 = persist.tile([1, E], fp32)
        addv = persist.tile([1, NCH * 2 * E], fp32)  # carry+offset per (tc,k)
        rppe_sb = persist.tile([1, E], fp32)
        ims_sb = persist.tile([1, E], fp32)

        # ---------------- layer 1 helper ----------------
        def layer1(w1_sb, nmc, bias_col):
            hb = wpool.tile([P, 8, TPC], fp32, tag="hbuf", name="hbuf")
            for mc in range(nmc):
                ps = l1ps.tile([P, TPC], fp32, tag="l1")
                for kc in range(8):
                    nc.tensor.matmul(out=ps[:], lhsT=w1_sb[:, kc, mc * P:(mc + 1) * P],
                                     rhs=xT[:, kc, :], start=(kc == 0), stop=(kc == 7))
                nc.scalar.activation(out=hb[:, mc, :], in_=ps[:], func=AF.Relu,
                                     bias=bias_col[:, mc:mc + 1], scale=1.0)
            return hb

        # --- importance net ---
        hbuf = layer1(wi1_sb, 4, bi1c)
        for tcch in range(NCH):
            zps = l2ps.tile([P, 1], fp32, tag="l2")
            for kc in range(4):
                nc.tensor.matmul(out=zps[:], lhsT=hbuf[:, kc, tcch * P:(tcch + 1) * P],
                                 rhs=wi2_sb[:, kc, :], start=(kc == 0), stop=False)
            nc.tensor.matmul(out=zps[:], lhsT=ones_row[:, :], rhs=bi2row[:, :],
                             start=False, stop=True)
            ts(mask_sb[:, tcch:tcch + 1], zps[:], 0.0, OP.is_gt)
            impv = sm.tile([P, 1], fp32, tag="impv")
            nc.scalar.activation(out=impv[:], in_=zps[:], func=AF.Sigmoid)
            nc.sync.dma_start(out=imp_d[tcch * P:(tcch + 1) * P, :], in_=impv[:])

        # --- router_important ---
        hbuf = layer1(wr1_sb, 8, br1c)
        for tcch in range(NCH):
            lps = l2ps.tile([P, E], fp32, tag="l2")
            for kc in range(8):
                nc.tensor.matmul(out=lps[:], lhsT=hbuf[:, kc, tcch * P:(tcch + 1) * P],
                                 rhs=wr2_sb[:, kc, :], start=(kc == 0), stop=False)
            nc.tensor.matmul(out=lps[:], lhsT=ones_row[:, :], rhs=br2row[:, :],
                             start=False, stop=True)
            nc.vector.tensor_copy(out=li_sb[:, tcch, :], in_=lps[:])

        # --- router_unimportant + decision chain (everything the counts
        #     AllGather needs: logits -> top2 -> one-hots -> counts) ---
        hbuf = layer1(wu1_sb, 8, bu1c)
        for tcch in range(NCH):
            tsl = slice(tcch * P, (tcch + 1) * P)
            lups = l2ps.tile([P, E], fp32, tag="l2")
            for kc in range(8):
                nc.tensor.matmul(out=lups[:], lhsT=hbuf[:, kc, tsl],
                                 rhs=wu2_sb[:, kc, :], start=(kc == 0), stop=False)
            nc.tensor.matmul(out=lups[:], lhsT=ones_row[:, :], rhs=bu2row[:, :],
                             start=False, stop=True)

            m = mask_sb[:, tcch:tcch + 1]
            invm = sm.tile([P, 1], fp32, tag="invm")
            ts(invm[:], m, -1.0, OP.mult, 1.0, OP.add)
            t1 = sm.tile([P, E], fp32, tag="t1")
            ts(t1[:], li_sb[:, tcch, :], m, OP.mult)
            t2 = sm.tile([P, E], fp32, tag="t2")
            ts(t2[:], lups[:], invm[:], OP.mult)
            tt(logits_sb[:, tcch, :], t1[:], t2[:], OP.add)
            logits = logits_sb[:, tcch, :]

            # top-2 on logits (exact fp32)
            rmax = rmax_sb[:, tcch:tcch + 1]
            nc.vector.tensor_reduce(out=rmax, in_=logits, axis=AX.X, op=OP.max)
            eq1 = sm.tile([P, E], fp32, tag="eq1")
            ts(eq1[:], logits, rmax, OP.is_equal)
            b1 = sm.tile([P, E], fp32, tag="b1")
            ts(b1[:], eq1[:], -999.0, OP.mult, 999.0, OP.add)
            c1 = sm.tile([P, E], fp32, tag="c1")
            tt(c1[:], b1[:], iota8[:], OP.add)
            nc.vector.tensor_reduce(out=idx_sb[:, tcch, 0:1], in_=c1[:], axis=AX.X,
                                    op=OP.min)
            ts(ohs[:, tcch, 0, :], iota8[:], idx_sb[:, tcch, 0:1], OP.is_equal)
            negm = sm.tile([P, E], fp32, tag="negm")
            ts(negm[:], ohs[:, tcch, 0, :], -1.0e9, OP.mult)
            lm = sm.tile([P, E], fp32, tag="lm")
            tt(lm[:], logits, negm[:], OP.add)
            l2v = sm.tile([P, 1], fp32, tag="l2v")
            nc.vector.tensor_reduce(out=l2v[:], in_=lm[:], axis=AX.X, op=OP.max)
            eq2 = sm.tile([P, E], fp32, tag="eq2")
            ts(eq2[:], lm[:], l2v[:], OP.is_equal)
            b2 = sm.tile([P, E], fp32, tag="b2")
            ts(b2[:], eq2[:], -999.0, OP.mult, 999.0, OP.add)
            c2 = sm.tile([P, E], fp32, tag="c2")
            tt(c2[:], b2[:], iota8[:], OP.add)
            nc.vector.tensor_reduce(out=idx_sb[:, tcch, 1:2], in_=c2[:], axis=AX.X,
                                    op=OP.min)
            ts(ohs[:, tcch, 1, :], iota8[:], idx_sb[:, tcch, 1:2], OP.is_equal)

            for k in range(2):
                ccp = ccps_p.tile([1, E], fp32, tag="ccp")
                nc.tensor.matmul(out=ccp[:], lhsT=ones_col[:, :],
                                 rhs=ohs[:, tcch, k, :], start=True, stop=True)
                nc.vector.tensor_copy(out=chcnt[:, k, tcch, :], in_=ccp[:])

        # totals + carries, then fire the counts AllGather ASAP
        for k in range(2):
            nc.vector.tensor_copy(out=carr[:, k, 1, :], in_=chcnt[:, k, 0, :])
            tt(carr[:, k, 2, :], carr[:, k, 1, :], chcnt[:, k, 1, :], OP.add)
            tt(carr[:, k, 3, :], carr[:, k, 2, :], chcnt[:, k, 2, :], OP.add)
            tt(agin[:, k * E:(k + 1) * E], carr[:, k, 3, :], chcnt[:, k, 3, :], OP.add)

        ag_in_d = dram.tile([1, 2 * E], fp32)
        ag_out_d = dram.tile([NCORES, 2 * E], fp32, addr_space="Shared")
        nc.gpsimd.dma_start(out=ag_in_d[:], in_=agin[:])
        nc.gpsimd.collective_compute(
            "AllGather", mybir.AluOpType.bypass,
            ins=[ag_in_d[:]], outs=[ag_out_d[:]], replica_groups=RG,
        )
        nc.gpsimd.dma_start(out=cnt_all[:], in_=ag_out_d[:])

        # --- value chain per chunk (overlaps the AllGather): softmax,
        #     gate weights, masked-prob sums, prefix matmuls ---
        for tcch in range(NCH):
            tsl = slice(tcch * P, (tcch + 1) * P)
            logits = logits_sb[:, tcch, :]
            nrmax = sm.tile([P, 1], fp32, tag="nrmax")
            ts(nrmax[:], rmax_sb[:, tcch:tcch + 1], -1.0, OP.mult)
            exp_t = sm.tile([P, E], fp32, tag="exp_t")
            ssum = sm.tile([P, 1], fp32, tag="ssum")
            nc.scalar.activation(out=exp_t[:], in_=logits, func=AF.Exp,
                                 bias=nrmax[:], scale=1.0, accum_out=ssum[:])
            rs0 = sm.tile([P, 1], fp32, tag="rs0")
            nc.vector.reciprocal(out=rs0[:], in_=ssum[:])
            nt1 = sm.tile([P, 1], fp32, tag="nt1")
            tt(nt1[:], ssum[:], rs0[:], OP.mult)
            nt2 = sm.tile([P, 1], fp32, tag="nt2")
            ts(nt2[:], nt1[:], -1.0, OP.mult, 2.0, OP.add)
            rs = sm.tile([P, 1], fp32, tag="rs")
            tt(rs[:], rs0[:], nt2[:], OP.mult)
            ts(probs_sb[:, tcch, :], exp_t[:], rs[:], OP.mult)
            nc.sync.dma_start(out=prob_d[tsl, :], in_=probs_sb[:, tcch, :])

            scr1 = sm.tile([P, E], fp32, tag="scr1")
            p1 = sm.tile([P, 1], fp32, tag="p1")
            tt(scr1[:], probs_sb[:, tcch, :], ohs[:, tcch, 0, :], OP.mult)
            nc.vector.tensor_reduce(out=p1[:], in_=scr1[:], axis=AX.X, op=OP.add)
            scr2 = sm.tile([P, E], fp32, tag="scr2")
            p2 = sm.tile([P, 1], fp32, tag="p2")
            tt(scr2[:], probs_sb[:, tcch, :], ohs[:, tcch, 1, :], OP.mult)
            nc.vector.tensor_reduce(out=p2[:], in_=scr2[:], axis=AX.X, op=OP.add)
            ws = sm.tile([P, 1], fp32, tag="ws")
            tt(ws[:], p1[:], p2[:], OP.add)
            rw0 = sm.tile([P, 1], fp32, tag="rw0")
            nc.vector.reciprocal(out=rw0[:], in_=ws[:])
            wt1 = sm.tile([P, 1], fp32, tag="wt1")
            tt(wt1[:], ws[:], rw0[:], OP.mult)
            wt2 = sm.tile([P, 1], fp32, tag="wt2")
            ts(wt2[:], wt1[:], -1.0, OP.mult, 2.0, OP.add)
            rw = sm.tile([P, 1], fp32, tag="rw")
            tt(rw[:], rw0[:], wt2[:], OP.mult)
            tt(w_sb[:, tcch, 0:1], p1[:], rw[:], OP.mult)
            tt(w_sb[:, tcch, 1:2], p2[:], rw[:], OP.mult)

            pm = sm.tile([P, E], fp32, tag="pm")
            ts(pm[:], probs_sb[:, tcch, :], mask_sb[:, tcch:tcch + 1], OP.mult)
            pst = ccps_p.tile([1, E], fp32, tag="ccp")
            nc.tensor.matmul(out=pst[:], lhsT=ones_col[:, :],
                             rhs=probs_sb[:, tcch, :], start=True, stop=True)
            if tcch == 0:
                nc.vector.tensor_copy(out=probsum_sb[:], in_=pst[:])
            else:
                tt(probsum_sb[:], probsum_sb[:], pst[:], OP.add)
            ist = ccps_p.tile([1, E], fp32, tag="ccp")
            nc.tensor.matmul(out=ist[:], lhsT=ones_col[:, :],
                             rhs=pm[:], start=True, stop=True)
            if tcch == 0:
                nc.vector.tensor_copy(out=impsum_sb[:], in_=ist[:])
            else:
                tt(impsum_sb[:], impsum_sb[:], ist[:], OP.add)

            for k in range(2):
                pfx = l2ps.tile([P, E], fp32, tag="l2")
                nc.tensor.matmul(out=pfx[:], lhsT=ut[:, :],
                                 rhs=ohs[:, tcch, k, :], start=True, stop=True)
                nc.vector.tensor_copy(out=prefix_sb[:, tcch, k, :], in_=pfx[:])

        # second AllGather for the aux partial sums (overlaps scatter tail)
        nc.vector.tensor_copy(out=agin2[:, 0:E], in_=probsum_sb[:])
        nc.vector.tensor_copy(out=agin2[:, E:2 * E], in_=impsum_sb[:])
        ag2_in_d = dram.tile([1, 2 * E], fp32)
        ag2_out_d = dram.tile([NCORES, 2 * E], fp32, addr_space="Shared")
        nc.gpsimd.dma_start(out=ag2_in_d[:], in_=agin2[:])
        nc.gpsimd.collective_compute(
            "AllGather", mybir.AluOpType.bypass,
            ins=[ag2_in_d[:]], outs=[ag2_out_d[:]], replica_groups=RG,
        )
        nc.gpsimd.dma_start(out=aux_all[:], in_=ag2_out_d[:])

        # cross-core offsets from the counts AG
        ms0 = ccps_p.tile([1, E], fp32, tag="ccp")
        nc.tensor.matmul(out=ms0[:], lhsT=cmask_sb[:, :], rhs=cnt_all[:, 0:E],
                         start=True, stop=True)
        nc.vector.tensor_copy(out=off0[:], in_=ms0[:])
        t0p = ccps_p.tile([1, E], fp32, tag="ccp")
        nc.tensor.matmul(out=t0p[:], lhsT=ones8[:, :], rhs=cnt_all[:, 0:E],
                         start=True, stop=True)
        nc.vector.tensor_copy(out=tot0[:], in_=t0p[:])
        ms1 = ccps_p.tile([1, E], fp32, tag="ccp")
        nc.tensor.matmul(out=ms1[:], lhsT=cmask_sb[:, :], rhs=cnt_all[:, E:2 * E],
                         start=True, stop=True)
        tt(off1[:], ms1[:], tot0[:], OP.add)

        for tcch in range(NCH):
            for k in range(2):
                sl = (tcch * 2 + k) * E
                offk = off0 if k == 0 else off1
                if tcch == 0:
                    nc.vector.tensor_copy(out=addv[:, sl:sl + E], in_=offk[:])
                else:
                    tt(addv[:, sl:sl + E], carr[:, k, tcch, :], offk[:], OP.add)
        bc_ps = l2ps.tile([P, NCH * 2 * E], fp32, tag="l2")
        nc.tensor.matmul(out=bc_ps[:], lhsT=ones_row[:, :], rhs=addv[:, :],
                         start=True, stop=True)

        # final positions + row scatter (comb rows built on gpsimd to
        # split the [P,CAP] elementwise work across two engines)
        for tcch in range(NCH):
            for k in range(2):
                sl = (tcch * 2 + k) * E
                padd = sm.tile([P, E], fp32, tag="padd")
                tt(padd[:], prefix_sb[:, tcch, k, :], bc_ps[:, sl:sl + E], OP.add)
                scr = sm.tile([P, E], fp32, tag="scr")
                posk = sm.tile([P, 1], fp32, tag="posk")
                tt(scr[:], padd[:], ohs[:, tcch, k, :], OP.mult)
                nc.vector.tensor_reduce(out=posk[:], in_=scr[:], axis=AX.X, op=OP.add)
                keep = sm.tile([P, 1], fp32, tag="keep")
                ts(keep[:], posk[:], float(CAP), OP.is_lt)
                posc = sm.tile([P, 1], fp32, tag="posc")
                ts(posc[:], posk[:], float(CAP - 1), OP.min)
                r0 = sm.tile([P, 1], fp32, tag="r0")
                ts(r0[:], iota_col8[:], float(tcch * P * E), OP.add)
                r1 = sm.tile([P, 1], fp32, tag="r1")
                tt(r1[:], r0[:], idx_sb[:, tcch, k:k + 1], OP.add)
                nk = sm.tile([P, 1], fp32, tag="nk")
                ts(nk[:], keep[:], -BIG, OP.mult, BIG, OP.add)
                rf = sm.tile([P, 1], fp32, tag="rf")
                tt(rf[:], r1[:], nk[:], OP.add)
                ri = sm.tile([P, 1], i32, tag="ri")
                nc.vector.tensor_copy(out=ri[:], in_=rf[:])

                ohrow = rowp.tile([P, CAP], fp32, tag="ohrow")
                ts(ohrow[:], iota_cap[:], posc[:], OP.is_equal)
                cmbrow = rowp.tile([P, CAP], fp32, tag="cmbrow")
                ts(cmbrow[:], iota_cap[:], posc[:], OP.is_equal, eng=nc.gpsimd)
                nc.gpsimd.tensor_scalar(out=cmbrow[:], in0=cmbrow[:],
                                        scalar1=w_sb[:, tcch, k:k + 1], scalar2=None,
                                        op0=OP.mult)

                s1 = nc.gpsimd.indirect_dma_start(
                    out=disp_d[:, :], out_offset=IOA(ap=ri[:, 0:1], axis=0),
                    in_=ohrow[:], in_offset=None,
                    bounds_check=TPC * E - 1, oob_is_err=False)
                s2 = nc.gpsimd.indirect_dma_start(
                    out=comb_d[:, :], out_offset=IOA(ap=ri[:, 0:1], axis=0),
                    in_=cmbrow[:], in_offset=None,
                    bounds_check=TPC * E - 1, oob_is_err=False)
                for z in zfill_insts["dispatch"]:
                    add_dep_helper(s1.ins, z.ins, reason="scatter after zero-fill")
                for z in zfill_insts["combine"]:
                    add_dep_helper(s2.ins, z.ins, reason="scatter after zero-fill")

        # ---------------- aux loss (from the aux AllGather) ----------------
        def ln_series(out_sb, d_sb, tagp):
            # ln(1+d) = d*(1 - d*(1/2 - d*(1/3 - ...)))
            s = persist.tile([1, E], fp32, name=f"{tagp}_s")
            nc.vector.memset(s[:], 1.0 / NTERMS)
            for i in range(NTERMS - 1, 0, -1):
                mtmp = sm.tile([1, E], fp32, tag=f"{tagp}_m")
                tt(mtmp[:], d_sb[:], s[:], OP.mult)
                ts(s[:], mtmp[:], -1.0, OP.mult, 1.0 / i, OP.add)
            tt(out_sb, d_sb[:], s[:], OP.mult)

        rpp = ccps_p.tile([1, E], fp32, tag="ccp")
        nc.tensor.matmul(out=rpp[:], lhsT=ones8[:, :], rhs=aux_all[:, 0:E],
                         start=True, stop=True)
        ts(rppe_sb[:], rpp[:], 1.0 / N, OP.mult)
        imt = ccps_p.tile([1, E], fp32, tag="ccp")
        nc.tensor.matmul(out=imt[:], lhsT=ones8[:, :], rhs=aux_all[:, E:2 * E],
                         start=True, stop=True)
        ts(ims_sb[:], imt[:], 1.0e-9, OP.add)

        t8 = persist.tile([1, E], fp32)
        ts(t8[:], rppe_sb[:], 8.0, OP.mult, 1.0e-9, OP.add)
        d1 = persist.tile([1, E], fp32)
        ts(d1[:], t8[:], -1.0, OP.add)
        ln1 = persist.tile([1, E], fp32)
        ln_series(ln1[:], d1, "lnA")
        elt = persist.tile([1, E], fp32)
        tt(elt[:], rppe_sb[:], ln1[:], OP.mult)
        el = persist.tile([1, 1], fp32)
        nc.vector.tensor_reduce(out=el[:], in_=elt[:], axis=AX.X, op=OP.add)

        st = persist.tile([1, 1], fp32)
        nc.vector.tensor_reduce(out=st[:], in_=ims_sb[:], axis=AX.X, op=OP.add)
        rst0 = persist.tile([1, 1], fp32)
        nc.vector.reciprocal(out=rst0[:], in_=st[:])
        at1 = persist.tile([1, 1], fp32)
        tt(at1[:], st[:], rst0[:], OP.mult)
        at2 = persist.tile([1, 1], fp32)
        ts(at2[:], at1[:], -1.0, OP.mult, 2.0, OP.add)
        rst = persist.tile([1, 1], fp32)
        tt(rst[:], rst0[:], at2[:], OP.mult)
        ippe = persist.tile([1, E], fp32)
        ts(ippe[:], ims_sb[:], rst[:], OP.mult)
        u8 = persist.tile([1, E], fp32)
        ts(u8[:], ippe[:], 8.0, OP.mult, 8.0e-9, OP.add)
        d2 = persist.tile([1, E], fp32)
        ts(d2[:], u8[:], -1.0, OP.add)
        ln2 = persist.tile([1, E], fp32)
        ln_series(ln2[:], d2, "lnB")
        lnip = persist.tile([1, E], fp32)
        ts(lnip[:], ln2[:], -LN8, OP.add)
        iet = persist.tile([1, E], fp32)
        tt(iet[:], ippe[:], lnip[:], OP.mult)
        ies = persist.tile([1, 1], fp32)
        nc.vector.tensor_reduce(out=ies[:], in_=iet[:], axis=AX.X, op=OP.add)
        # aux = el + (0.1/ln8) * sum(ippe*ln(ippe+eps))   [ies = -imp_entropy]
        sc = persist.tile([1, 1], fp32)
        ts(sc[:], ies[:], 0.1 / LN8, OP.mult)
        auxv = persist.tile([1, 1], fp32)
        tt(auxv[:], el[:], sc[:], OP.add)
        nc.sync.dma_start(out=aux_d[:, :], in_=auxv[:])

    nc.compile()
    return nc


def _get_nc():
    if "nc" not in _CACHE:
        _CACHE["nc"] = _build()
    return _CACHE["nc"]


def kernel(**inputs):
    global LAST_EXEC_NS, LAST_TRACE_DIR
    from concourse.bass_utils import run_bass_kernel_spmd

    inp = {k: np.ascontiguousarray(np.asarray(v), dtype=np.float32)
           for k, v in inputs.items()}
    x = inp["hidden_states"].reshape(N, H)

    nc = _get_nc()
    in_maps = []
    for c in range(NCORES):
        in_maps.append(dict(
            hidden_states=np.ascontiguousarray(x[c * TPC:(c + 1) * TPC]),
            wi1=inp["wi1"], bi1=inp["bi1"], wi2=inp["wi2"],
            bi2=inp["bi2"].reshape(1, 1),
            wr1=inp["wr1"], br1=inp["br1"], wr2=inp["wr2"],
            br2=inp["br2"].reshape(1, E),
            wu1=inp["wu1"], bu1=inp["bu1"], wu2=inp["wu2"],
            bu2=inp["bu2"].reshape(1, E),
            cmask=(np.arange(NCORES) < c).astype(np.float32).reshape(NCORES, 1),
        ))

    trace = bool(int(os.environ.get("KERNEL_TRACE", "0")))
    kwargs = {}
    if trace:
        _install_ntff_hook()
        import tempfile
        LAST_TRACE_DIR = tempfile.mkdtemp(prefix="adaptive_router_trace_")
        kwargs["tmpdir"] = LAST_TRACE_DIR
    res = run_bass_kernel_spmd(nc, in_maps, core_ids=list(range(NCORES)),
                               trace=trace, **kwargs)
    LAST_EXEC_NS = res.exec_time_ns

    disp = np.concatenate(
        [res.results[c]["dispatch"].reshape(TPC, E, CAP) for c in range(NCORES)], 0
    ).reshape(B, S, E, CAP)
    comb = np.concatenate(
        [res.results[c]["combine"].reshape(TPC, E, CAP) for c in range(NCORES)], 0
    ).reshape(B, S, E, CAP)
    probs = np.concatenate(
        [res.results[c]["probs"] for c in range(NCORES)], 0).reshape(B, S, E)
    impv = np.concatenate(
        [res.results[c]["importance"] for c in range(NCORES)], 0).reshape(B, S, 1)
    aux = np.float32(res.results[0]["aux"].reshape(()))
    return disp, comb, probs, aux, impv


# revision 15
# speedup vs baseline: 1.9226x; 1.9226x over previous
"""Trainium2 Bass kernel for nn_AdaptiveRouter (MoE dual-gate routing).

8 NeuronCores, data-parallel over tokens. Each core handles 512 tokens:
  - fp32 matmuls for importance MLP + both routers (decisions are
    numerically sensitive: mask threshold and top-k tie distances)
  - top-2 selection via exact fp32 compares on logits
  - capacity positions: strict-upper-triangular prefix matmuls within a
    128-token chunk, chunk carries, and an 8-core AllGather of per-core
    per-(k,expert) counts for the global k-major cumsum offsets
  - dispatch/combine [512,8,1536] f32 shards: zero-filled with large
    DMA stores, then one 1536-float row per (token,k) scattered via
    indirect DMA (OOB row index drops capacity-overflow entries)
  - second AllGather for aux-loss partial sums; ln() via series
"""

import math
import os
import sys
import types

sys.path.insert(0, "/opt/trn_rl_repo")

import numpy as np

# ---- problem constants (hardcoded; kernel.py must be self-contained) ----
B, S, H, E, TOPK, CF = 2, 2048, 1024, 8, 2, 1.5
N = B * S                      # 4096 tokens
NCORES = 8
TPC = N // NCORES              # 512 tokens per core
CAP = int(N * CF * TOPK / E)   # 1536
P = 128
NCH = TPC // P                 # 4 token chunks per core
H2 = H // 2
BIG = 1.0e6                    # row marker for dropped entries (OOB -> skipped)
LN8 = math.log(8.0)
NTERMS = 12                    # ln(1+x) series terms

LAST_EXEC_NS = None
LAST_TRACE_DIR = None

_CACHE = {}


def _install_ntff_hook():
    """Recreate antenv.axon_hooks (absent in this image) so
    run_bass_kernel_spmd(trace=True) can profile via libaxon_pjrt."""
    import antenv

    if "antenv.axon_hooks" not in sys.modules:
        mod = types.ModuleType("antenv.axon_hooks")
        mod._hook = None

        def set_axon_ntff_profile_hook(h):
            mod._hook = h

        def get_axon_ntff_profile_hook():
            return mod._hook

        mod.set_axon_ntff_profile_hook = set_axon_ntff_profile_hook
        mod.get_axon_ntff_profile_hook = get_axon_ntff_profile_hook
        sys.modules["antenv.axon_hooks"] = mod
        antenv.axon_hooks = mod
    mod = sys.modules["antenv.axon_hooks"]
    if mod._hook is None:
        from trn_agent_boot.trn_boot import _ntff_profile_via_ctypes

        mod.set_axon_ntff_profile_hook(
            _ntff_profile_via_ctypes("/opt/axon/libaxon_pjrt.so")
        )


def _build():
    import concourse.bacc as bacc
    import concourse.mybir as mybir
    import concourse.tile as tile
    import concourse.bass as bass
    from concourse.masks import make_identity, make_upper_triangular
    from concourse.tile import add_dep_helper
    from contextlib import ExitStack

    fp32 = mybir.dt.float32
    i32 = mybir.dt.int32
    AF = mybir.ActivationFunctionType
    OP = mybir.AluOpType
    AX = mybir.AxisListType

    nc = bacc.Bacc(None, target_bir_lowering=False)

    x_d = nc.declare_dram_parameter("hidden_states", [TPC, H], fp32, isOutput=False)
    wi1_d = nc.declare_dram_parameter("wi1", [H, H2], fp32, isOutput=False)
    bi1_d = nc.declare_dram_parameter("bi1", [H2], fp32, isOutput=False)
    wi2_d = nc.declare_dram_parameter("wi2", [H2, 1], fp32, isOutput=False)
    bi2_d = nc.declare_dram_parameter("bi2", [1, 1], fp32, isOutput=False)
    wr1_d = nc.declare_dram_parameter("wr1", [H, H], fp32, isOutput=False)
    br1_d = nc.declare_dram_parameter("br1", [H], fp32, isOutput=False)
    wr2_d = nc.declare_dram_parameter("wr2", [H, E], fp32, isOutput=False)
    br2_d = nc.declare_dram_parameter("br2", [1, E], fp32, isOutput=False)
    wu1_d = nc.declare_dram_parameter("wu1", [H, H], fp32, isOutput=False)
    bu1_d = nc.declare_dram_parameter("bu1", [H], fp32, isOutput=False)
    wu2_d = nc.declare_dram_parameter("wu2", [H, E], fp32, isOutput=False)
    bu2_d = nc.declare_dram_parameter("bu2", [1, E], fp32, isOutput=False)
    cm_d = nc.declare_dram_parameter("cmask", [NCORES, 1], fp32, isOutput=False)

    disp_d = nc.declare_dram_parameter("dispatch", [TPC * E, CAP], fp32, isOutput=True)
    comb_d = nc.declare_dram_parameter("combine", [TPC * E, CAP], fp32, isOutput=True)
    prob_d = nc.declare_dram_parameter("probs", [TPC, E], fp32, isOutput=True)
    imp_d = nc.declare_dram_parameter("importance", [TPC, 1], fp32, isOutput=True)
    aux_d = nc.declare_dram_parameter("aux", [1, 1], fp32, isOutput=True)

    IOA = bass.IndirectOffsetOnAxis
    RG = [list(range(NCORES))]

    with tile.TileContext(nc) as tc, ExitStack() as ctx:
        const = ctx.enter_context(tc.tile_pool(name="const", bufs=1))
        wpool = ctx.enter_context(tc.tile_pool(name="wpool", bufs=1))
        persist = ctx.enter_context(tc.tile_pool(name="persist", bufs=1))
        xraw_p = ctx.enter_context(tc.tile_pool(name="xraw", bufs=2))
        sm = ctx.enter_context(tc.tile_pool(name="sm", bufs=3))       # small transients
        rowp = ctx.enter_context(tc.tile_pool(name="rowp", bufs=3))   # [P,CAP] rows
        l1ps = ctx.enter_context(tc.tile_pool(name="l1ps", bufs=2, space="PSUM"))
        l2ps = ctx.enter_context(tc.tile_pool(name="l2ps", bufs=2, space="PSUM"))
        ccps_p = ctx.enter_context(tc.tile_pool(name="ccps", bufs=2, space="PSUM"))
        dram = ctx.enter_context(tc.tile_pool(name="dram", bufs=1, space="DRAM"))

        def ts(out, in0, s1, op0, s2=None, op1=None, eng=None):
            e = eng if eng is not None else nc.vector
            if s2 is None:
                e.tensor_scalar(out=out, in0=in0, scalar1=s1, scalar2=None, op0=op0)
            else:
                e.tensor_scalar(out=out, in0=in0, scalar1=s1, scalar2=s2,
                                op0=op0, op1=op1)

        def tt(out, a, b, op):
            nc.vector.tensor_tensor(out=out, in0=a, in1=b, op=op)

        # ---------------- constants ----------------
        ident = const.tile([P, P], fp32)
        make_identity(nc, ident[:])
        ut = const.tile([P, P], fp32)
        make_upper_triangular(nc, ut[:], val=1.0, diag=False)  # strictly upper
        iota8 = const.tile([P, E], fp32)
        nc.gpsimd.iota(iota8[:], pattern=[[1, E]], base=0, channel_multiplier=0,
                       allow_small_or_imprecise_dtypes=True)
        iota_col8 = const.tile([P, 1], fp32)
        nc.gpsimd.iota(iota_col8[:], pattern=[[0, 1]], base=0, channel_multiplier=E,
                       allow_small_or_imprecise_dtypes=True)  # p*8
        iota_cap = const.tile([P, CAP], fp32)
        nc.gpsimd.iota(iota_cap[:], pattern=[[1, CAP]], base=0, channel_multiplier=0,
                       allow_small_or_imprecise_dtypes=True)
        ones_row = const.tile([1, P], fp32)
        nc.vector.memset(ones_row[:], 1.0)
        ones_col = const.tile([P, 1], fp32)
        nc.vector.memset(ones_col[:], 1.0)
        ones8 = const.tile([NCORES, 1], fp32)
        nc.vector.memset(ones8[:], 1.0)
        zero_sb = const.tile([P, 2048], fp32)
        nc.vector.memset(zero_sb[:], 0.0)

        # ---------------- x load + transpose (PE critical path first) -----
        xT = wpool.tile([P, 8, TPC], fp32)
        for tcch in range(NCH):
            x_raw = xraw_p.tile([P, H], fp32, tag="xraw")
            nc.sync.dma_start(out=x_raw[:], in_=x_d[tcch * P:(tcch + 1) * P, :])
            for kc in range(8):
                tps = l2ps.tile([P, P], fp32, tag="l2")
                nc.tensor.transpose(out=tps[:], in_=x_raw[:, kc * P:(kc + 1) * P],
                                    identity=ident[:])
                nc.vector.tensor_copy(out=xT[:, kc, tcch * P:(tcch + 1) * P], in_=tps[:])

        # ---------------- weight loads ----------------
        wi1_sb = wpool.tile([P, 8, H2], fp32)
        nc.sync.dma_start(out=wi1_sb[:], in_=wi1_d[:, :].rearrange("(k p) m -> p k m", p=P))
        wr1_sb = wpool.tile([P, 8, H], fp32)
        nc.sync.dma_start(out=wr1_sb[:], in_=wr1_d[:, :].rearrange("(k p) m -> p k m", p=P))
        wu1_sb = wpool.tile([P, 8, H], fp32)
        nc.sync.dma_start(out=wu1_sb[:], in_=wu1_d[:, :].rearrange("(k p) m -> p k m", p=P))
        wi2_sb = wpool.tile([P, 4, 1], fp32)
        nc.sync.dma_start(out=wi2_sb[:], in_=wi2_d[:, :].rearrange("(k p) m -> p k m", p=P))
        wr2_sb = wpool.tile([P, 8, E], fp32)
        nc.sync.dma_start(out=wr2_sb[:], in_=wr2_d[:, :].rearrange("(k p) m -> p k m", p=P))
        wu2_sb = wpool.tile([P, 8, E], fp32)
        nc.sync.dma_start(out=wu2_sb[:], in_=wu2_d[:, :].rearrange("(k p) m -> p k m", p=P))
        bi1c = wpool.tile([P, 4], fp32)
        nc.sync.dma_start(out=bi1c[:], in_=bi1_d[:].rearrange("(m p) -> p m", p=P))
        br1c = wpool.tile([P, 8], fp32)
        nc.sync.dma_start(out=br1c[:], in_=br1_d[:].rearrange("(m p) -> p m", p=P))
        bu1c = wpool.tile([P, 8], fp32)
        nc.sync.dma_start(out=bu1c[:], in_=bu1_d[:].rearrange("(m p) -> p m", p=P))
        br2row = wpool.tile([1, E], fp32)
        nc.sync.dma_start(out=br2row[:], in_=br2_d[:, :])
        bu2row = wpool.tile([1, E], fp32)
        nc.sync.dma_start(out=bu2row[:], in_=bu2_d[:, :])
        bi2row = wpool.tile([1, 1], fp32)
        nc.sync.dma_start(out=bi2row[:], in_=bi2_d[:, :])
        cmask_sb = wpool.tile([NCORES, 1], fp32)
        nc.sync.dma_start(out=cmask_sb[:], in_=cm_d[:, :])

        # ---------------- zero-fill dispatch/combine (fills DMA idle) -----
        zfill_insts = {"dispatch": [], "combine": []}
        for name, dd in (("dispatch", disp_d), ("combine", comb_d)):
            flat = dd[:, :].rearrange("r c -> (r c)")
            chunk = P * 2048
            for i in range((TPC * E * CAP) // chunk):   # 24 x 1MiB each
                ins = nc.sync.dma_start(
                    out=flat[i * chunk:(i + 1) * chunk].rearrange("(p f) -> p f", p=P),
                    in_=zero_sb[:],
                )
                zfill_insts[name].append(ins)

        # ---------------- persistent small tensors ----------------
        mask_sb = persist.tile([P, NCH], fp32)
        li_sb = persist.tile([P, NCH, E], fp32)
        logits_sb = persist.tile([P, NCH, E], fp32)
        rmax_sb = persist.tile([P, NCH], fp32)
        probs_sb = persist.tile([P, NCH, E], fp32)
        ohs = persist.tile([P, NCH, 2, E], fp32)
        idx_sb = persist.tile([P, NCH, 2], fp32)
        w_sb = persist.tile([P, NCH, 2], fp32)
        prefix_sb = persist.tile([P, NCH, 2, E], fp32)  # within-chunk excl prefix
        chcnt = persist.tile([1, 2, NCH, E], fp32)   # per (k, chunk) counts
        carr = persist.tile([1, 2, NCH, E], fp32)    # per (k, chunk) carry
        probsum_sb = persist.tile([1, E], fp32)
        impsum_sb = persist.tile([1, E], fp32)
        agin = persist.tile([1, 2 * E], fp32)        # counts AG payload
        agin2 = persist.tile([1, 2 * E], fp32)       # aux AG payload
        cnt_all = persist.tile([NCORES, 2 * E], fp32)
        aux_all = persist.tile([NCORES, 2 * E], fp32)
        off0 = persist.tile([1, E], fp32)
        off1 = persist.tile([1, E], fp32)
        tot0 = persist.tile([1, E], fp32)
        addv = persist.tile([1, NCH * 2 * E], fp32)  # carry+offset per (tc,k)
        rppe_sb = persist.tile([1, E], fp32)
        ims_sb = persist.tile([1, E], fp32)

        # ---------------- layer 1 helper ----------------
        def layer1(w1_sb, nmc, bias_col):
            hb = wpool.tile([P, 8, TPC], fp32, tag="hbuf", name="hbuf")
            for mc in range(nmc):
                ps = l1ps.tile([P, TPC], fp32, tag="l1")
                for kc in range(8):
                    nc.tensor.matmul(out=ps[:], lhsT=w1_sb[:, kc, mc * P:(mc + 1) * P],
                                     rhs=xT[:, kc, :], start=(kc == 0), stop=(kc == 7))
                nc.scalar.activation(out=hb[:, mc, :], in_=ps[:], func=AF.Relu,
                                     bias=bias_col[:, mc:mc + 1], scale=1.0)
            return hb

        # --- importance net ---
        hbuf = layer1(wi1_sb, 4, bi1c)
        for tcch in range(NCH):
            zps = l2ps.tile([P, 1], fp32, tag="l2")
            for kc in range(4):
                nc.tensor.matmul(out=zps[:], lhsT=hbuf[:, kc, tcch * P:(tcch + 1) * P],
                                 rhs=wi2_sb[:, kc, :], start=(kc == 0), stop=False)
            nc.tensor.matmul(out=zps[:], lhsT=ones_row[:, :], rhs=bi2row[:, :],
                             start=False, stop=True)
            ts(mask_sb[:, tcch:tcch + 1], zps[:], 0.0, OP.is_gt)
            impv = sm.tile([P, 1], fp32, tag="impv")
            nc.scalar.activation(out=impv[:], in_=zps[:], func=AF.Sigmoid)
            nc.sync.dma_start(out=imp_d[tcch * P:(tcch + 1) * P, :], in_=impv[:])

        # --- router_important ---
        hbuf = layer1(wr1_sb, 8, br1c)
        for tcch in range(NCH):
            lps = l2ps.tile([P, E], fp32, tag="l2")
            for kc in range(8):
                nc.tensor.matmul(out=lps[:], lhsT=hbuf[:, kc, tcch * P:(tcch + 1) * P],
                                 rhs=wr2_sb[:, kc, :], start=(kc == 0), stop=False)
            nc.tensor.matmul(out=lps[:], lhsT=ones_row[:, :], rhs=br2row[:, :],
                             start=False, stop=True)
            nc.vector.tensor_copy(out=li_sb[:, tcch, :], in_=lps[:])

        # --- router_unimportant + decision chain (everything the counts
        #     AllGather needs: logits -> top2 -> one-hots -> counts) ---
        hbuf = layer1(wu1_sb, 8, bu1c)
        for tcch in range(NCH):
            tsl = slice(tcch * P, (tcch + 1) * P)
            lups = l2ps.tile([P, E], fp32, tag="l2")
            for kc in range(8):
                nc.tensor.matmul(out=lups[:], lhsT=hbuf[:, kc, tsl],
                                 rhs=wu2_sb[:, kc, :], start=(kc == 0), stop=False)
            nc.tensor.matmul(out=lups[:], lhsT=ones_row[:, :], rhs=bu2row[:, :],
                             start=False, stop=True)

            m = mask_sb[:, tcch:tcch + 1]
            invm = sm.tile([P, 1], fp32, tag="invm")
            ts(invm[:], m, -1.0, OP.mult, 1.0, OP.add)
            t1 = sm.tile([P, E], fp32, tag="t1")
            ts(t1[:], li_sb[:, tcch, :], m, OP.mult)
            t2 = sm.tile([P, E], fp32, tag="t2")
            ts(t2[:], lups[:], invm[:], OP.mult)
            tt(logits_sb[:, tcch, :], t1[:], t2[:], OP.add)
            logits = logits_sb[:, tcch, :]

            # top-2 on logits (exact fp32)
            rmax = rmax_sb[:, tcch:tcch + 1]
            nc.vector.tensor_reduce(out=rmax, in_=logits, axis=AX.X, op=OP.max)
            eq1 = sm.tile([P, E], fp32, tag="eq1")
            ts(eq1[:], logits, rmax, OP.is_equal)
            b1 = sm.tile([P, E], fp32, tag="b1")
            ts(b1[:], eq1[:], -999.0, OP.mult, 999.0, OP.add)
            c1 = sm.tile([P, E], fp32, tag="c1")
            tt(c1[:], b1[:], iota8[:], OP.add)
            nc.vector.tensor_reduce(out=idx_sb[:, tcch, 0:1], in_=c1[:], axis=AX.X,
                                    op=OP.min)
            ts(ohs[:, tcch, 0, :], iota8[:], idx_sb[:, tcch, 0:1], OP.is_equal)
            negm = sm.tile([P, E], fp32, tag="negm")
            ts(negm[:], ohs[:, tcch, 0, :], -1.0e9, OP.mult)
            lm = sm.tile([P, E], fp32, tag="lm")
            tt(lm[:], logits, negm[:], OP.add)
            l2v = sm.tile([P, 1], fp32, tag="l2v")
            nc.vector.tensor_reduce(out=l2v[:], in_=lm[:], axis=AX.X, op=OP.max)
            eq2 = sm.tile([P, E], fp32, tag="eq2")
            ts(eq2[:], lm[:], l2v[:], OP.is_equal)
            b2 = sm.tile([P, E], fp32, tag="b2")
            ts(b2[:], eq2[:], -999.0, OP.mult, 999.0, OP.add)
            c2 = sm.tile([P, E], fp32, tag="c2")
            tt(c2[:], b2[:], iota8[:], OP.add)
            nc.vector.tensor_reduce(out=idx_sb[:, tcch, 1:2], in_=c2[:], axis=AX.X,
                                    op=OP.min)
            ts(ohs[:, tcch, 1, :], iota8[:], idx_sb[:, tcch, 1:2], OP.is_equal)

            for k in range(2):
                ccp = ccps_p.tile([1, E], fp32, tag="ccp")
                nc.tensor.matmul(out=ccp[:], lhsT=ones_col[:, :],
                                 rhs=ohs[:, tcch, k, :], start=True, stop=True)
                nc.vector.tensor_copy(out=chcnt[:, k, tcch, :], in_=ccp[:])

        # totals + carries, then fire the counts AllGather ASAP
        for k in range(2):
            nc.vector.tensor_copy(out=carr[:, k, 1, :], in_=chcnt[:, k, 0, :])
            tt(carr[:, k, 2, :], carr[:, k, 1, :], chcnt[:, k, 1, :], OP.add)
            tt(carr[:, k, 3, :], carr[:, k, 2, :], chcnt[:, k, 2, :], OP.add)
            tt(agin[:, k * E:(k + 1) * E], carr[:, k, 3, :], chcnt[:, k, 3, :], OP.add)

        ag_in_d = dram.tile([1, 2 * E], fp32)
        ag_out_d = dram.tile([NCORES, 2 * E], fp32, addr_space="Shared")
        nc.gpsimd.dma_start(out=ag_in_d[:], in_=agin[:])
        nc.gpsimd.collective_compute(
            "AllGather", mybir.AluOpType.bypass,
            ins=[ag_in_d[:]], outs=[ag_out_d[:]], replica_groups=RG,
        )
        nc.gpsimd.dma_start(out=cnt_all[:], in_=ag_out_d[:])

        # --- value chain per chunk (overlaps the AllGather): softmax,
        #     gate weights, masked-prob sums, prefix matmuls ---
        for tcch in range(NCH):
            tsl = slice(tcch * P, (tcch + 1) * P)
            logits = logits_sb[:, tcch, :]
            nrmax = sm.tile([P, 1], fp32, tag="nrmax")
            ts(nrmax[:], rmax_sb[:, tcch:tcch + 1], -1.0, OP.mult)
            exp_t = sm.tile([P, E], fp32, tag="exp_t")
            ssum = sm.tile([P, 1], fp32, tag="ssum")
            nc.scalar.activation(out=exp_t[:], in_=logits, func=AF.Exp,
                                 bias=nrmax[:], scale=1.0, accum_out=ssum[:])
            rs0 = sm.tile([P, 1], fp32, tag="rs0")
            nc.vector.reciprocal(out=rs0[:], in_=ssum[:])
            nt1 = sm.tile([P, 1], fp32, tag="nt1")
            tt(nt1[:], ssum[:], rs0[:], OP.mult)
            nt2 = sm.tile([P, 1], fp32, tag="nt2")
            ts(nt2[:], nt1[:], -1.0, OP.mult, 2.0, OP.add)
            rs = sm.tile([P, 1], fp32, tag="rs")
            tt(rs[:], rs0[:], nt2[:], OP.mult)
            ts(probs_sb[:, tcch, :], exp_t[:], rs[:], OP.mult)
            nc.sync.dma_start(out=prob_d[tsl, :], in_=probs_sb[:, tcch, :])

            scr1 = sm.tile([P, E], fp32, tag="scr1")
            p1 = sm.tile([P, 1], fp32, tag="p1")
            tt(scr1[:], probs_sb[:, tcch, :], ohs[:, tcch, 0, :], OP.mult)
            nc.vector.tensor_reduce(out=p1[:], in_=scr1[:], axis=AX.X, op=OP.add)
            scr2 = sm.tile([P, E], fp32, tag="scr2")
            p2 = sm.tile([P, 1], fp32, tag="p2")
            tt(scr2[:], probs_sb[:, tcch, :], ohs[:, tcch, 1, :], OP.mult)
            nc.vector.tensor_reduce(out=p2[:], in_=scr2[:], axis=AX.X, op=OP.add)
            ws = sm.tile([P, 1], fp32, tag="ws")
            tt(ws[:], p1[:], p2[:], OP.add)
            rw0 = sm.tile([P, 1], fp32, tag="rw0")
            nc.vector.reciprocal(out=rw0[:], in_=ws[:])
            wt1 = sm.tile([P, 1], fp32, tag="wt1")
            tt(wt1[:], ws[:], rw0[:], OP.mult)
            wt2 = sm.tile([P, 1], fp32, tag="wt2")
            ts(wt2[:], wt1[:], -1.0, OP.mult, 2.0, OP.add)
            rw = sm.tile([P, 1], fp32, tag="rw")
            tt(rw[:], rw0[:], wt2[:], OP.mult)
            tt(w_sb[:, tcch, 0:1], p1[:], rw[:], OP.mult)
            tt(w_sb[:, tcch, 1:2], p2[:], rw[:], OP.mult)

            pm = sm.tile([P, E], fp32, tag="pm")
            ts(pm[:], probs_sb[:, tcch, :], mask_sb[:, tcch:tcch + 1], OP.mult)
            pst = ccps_p.tile([1, E], fp32, tag="ccp")
            nc.tensor.matmul(out=pst[:], lhsT=ones_col[:, :],
                             rhs=probs_sb[:, tcch, :], start=True, stop=True)
            if tcch == 0:
                nc.vector.tensor_copy(out=probsum_sb[:], in_=pst[:])
            else:
                tt(probsum_sb[:], probsum_sb[:], pst[:], OP.add)
            ist = ccps_p.tile([1, E], fp32, tag="ccp")
            nc.tensor.matmul(out=ist[:], lhsT=ones_col[:, :],
                             rhs=pm[:], start=True, stop=True)
            if tcch == 0:
                nc.vector.tensor_copy(out=impsum_sb[:], in_=ist[:])
            else:
                tt(impsum_sb[:], impsum_sb[:], ist[:], OP.add)

            for k in range(2):
                pfx = l2ps.tile([P, E], fp32, tag="l2")
                nc.tensor.matmul(out=pfx[:], lhsT=ut[:, :],
                                 rhs=ohs[:, tcch, k, :], start=True, stop=True)
                nc.vector.tensor_copy(out=prefix_sb[:, tcch, k, :], in_=pfx[:])

        # second AllGather for the aux partial sums (overlaps scatter tail)
        nc.vector.tensor_copy(out=agin2[:, 0:E], in_=probsum_sb[:])
        nc.vector.tensor_copy(out=agin2[:, E:2 * E], in_=impsum_sb[:])
        ag2_in_d = dram.tile([1, 2 * E], fp32)
        ag2_out_d = dram.tile([NCORES, 2 * E], fp32, addr_space="Shared")
        nc.gpsimd.dma_start(out=ag2_in_d[:], in_=agin2[:])
        nc.gpsimd.collective_compute(
            "AllGather", mybir.AluOpType.bypass,
            ins=[ag2_in_d[:]], outs=[ag2_out_d[:]], replica_groups=RG,
        )
        nc.gpsimd.dma_start(out=aux_all[:], in_=ag2_out_d[:])

        # cross-core offsets from the counts AG
        ms0 = ccps_p.tile([1, E], fp32, tag="ccp")
        nc.tensor.matmul(out=ms0[:], lhsT=cmask_sb[:, :], rhs=cnt_all[:, 0:E],
                         start=True, stop=True)
        nc.vector.tensor_copy(out=off0[:], in_=ms0[:])
        t0p = ccps_p.tile([1, E], fp32, tag="ccp")
        nc.tensor.matmul(out=t0p[:], lhsT=ones8[:, :], rhs=cnt_all[:, 0:E],
                         start=True, stop=True)
        nc.vector.tensor_copy(out=tot0[:], in_=t0p[:])
        ms1 = ccps_p.tile([1, E], fp32, tag="ccp")
        nc.tensor.matmul(out=ms1[:], lhsT=cmask_sb[:, :], rhs=cnt_all[:, E:2 * E],
                         start=True, stop=True)
        tt(off1[:], ms1[:], tot0[:], OP.add)

        for tcch in range(NCH):
            for k in range(2):
                sl = (tcch * 2 + k) * E
                offk = off0 if k == 0 else off1
                if tcch == 0:
                    nc.vector.tensor_copy(out=addv[:, sl:sl + E], in_=offk[:])
                else:
                    tt(addv[:, sl:sl + E], carr[:, k, tcch, :], offk[:], OP.add)
        bc_ps = l2ps.tile([P, NCH * 2 * E], fp32, tag="l2")
        nc.tensor.matmul(out=bc_ps[:], lhsT=ones_row[:, :], rhs=addv[:, :],
                         start=True, stop=True)

        # final positions + row scatter (comb rows built on gpsimd to
        # split the [P,CAP] elementwise work across two engines)
        for tcch in range(NCH):
            for k in range(2):
                sl = (tcch * 2 + k) * E
                padd = sm.tile([P, E], fp32, tag="padd")
                tt(padd[:], prefix_sb[:, tcch, k, :], bc_ps[:, sl:sl + E], OP.add)
                scr = sm.tile([P, E], fp32, tag="scr")
                posk = sm.tile([P, 1], fp32, tag="posk")
                tt(scr[:], padd[:], ohs[:, tcch, k, :], OP.mult)
                nc.vector.tensor_reduce(out=posk[:], in_=scr[:], axis=AX.X, op=OP.add)
                keep = sm.tile([P, 1], fp32, tag="keep")
                ts(keep[:], posk[:], float(CAP), OP.is_lt)
                posc = sm.tile([P, 1], fp32, tag="posc")
                ts(posc[:], posk[:], float(CAP - 1), OP.min)
                r0 = sm.tile([P, 1], fp32, tag="r0")
                ts(r0[:], iota_col8[:], float(tcch * P * E), OP.add)
                r1 = sm.tile([P, 1], fp32, tag="r1")
                tt(r1[:], r0[:], idx_sb[:, tcch, k:k + 1], OP.add)
                nk = sm.tile([P, 1], fp32, tag="nk")
                ts(nk[:], keep[:], -BIG, OP.mult, BIG, OP.add)
                rf = sm.tile([P, 1], fp32, tag="rf")
                tt(rf[:], r1[:], nk[:], OP.add)
                ri = sm.tile([P, 1], i32, tag="ri")
                nc.vector.tensor_copy(out=ri[:], in_=rf[:])

                ohrow = rowp.tile([P, CAP], fp32, tag="ohrow")
                ts(ohrow[:], iota_cap[:], posc[:], OP.is_equal)
                cmbrow = rowp.tile([P, CAP], fp32, tag="cmbrow")
                ts(cmbrow[:], ohrow[:], w_sb[:, tcch, k:k + 1], OP.mult)

                s1 = nc.gpsimd.indirect_dma_start(
                    out=disp_d[:, :], out_offset=IOA(ap=ri[:, 0:1], axis=0),
                    in_=ohrow[:], in_offset=None,
                    bounds_check=TPC * E - 1, oob_is_err=False)
                s2 = nc.gpsimd.indirect_dma_start(
                    out=comb_d[:, :], out_offset=IOA(ap=ri[:, 0:1], axis=0),
                    in_=cmbrow[:], in_offset=None,
                    bounds_check=TPC * E - 1, oob_is_err=False)
                for z in zfill_insts["dispatch"]:
                    add_dep_helper(s1.ins, z.ins, reason="scatter after zero-fill")
                for z in zfill_insts["combine"]:
                    add_dep_helper(s2.ins, z.ins, reason="scatter after zero-fill")

        # ---------------- aux loss (from the aux AllGather) ----------------
        def ln_series(out_sb, d_sb, tagp):
            # ln(1+d) = d*(1 - d*(1/2 - d*(1/3 - ...)))
            s = persist.tile([1, E], fp32, name=f"{tagp}_s")
            nc.vector.memset(s[:], 1.0 / NTERMS)
            for i in range(NTERMS - 1, 0, -1):
                mtmp = sm.tile([1, E], fp32, tag=f"{tagp}_m")
                tt(mtmp[:], d_sb[:], s[:], OP.mult)
                ts(s[:], mtmp[:], -1.0, OP.mult, 1.0 / i, OP.add)
            tt(out_sb, d_sb[:], s[:], OP.mult)

        rpp = ccps_p.tile([1, E], fp32, tag="ccp")
        nc.tensor.matmul(out=rpp[:], lhsT=ones8[:, :], rhs=aux_all[:, 0:E],
                         start=True, stop=True)
        ts(rppe_sb[:], rpp[:], 1.0 / N, OP.mult)
        imt = ccps_p.tile([1, E], fp32, tag="ccp")
        nc.tensor.matmul(out=imt[:], lhsT=ones8[:, :], rhs=aux_all[:, E:2 * E],
                         start=True, stop=True)
        ts(ims_sb[:], imt[:], 1.0e-9, OP.add)

        t8 = persist.tile([1, E], fp32)
        ts(t8[:], rppe_sb[:], 8.0, OP.mult, 1.0e-9, OP.add)
        d1 = persist.tile([1, E], fp32)
        ts(d1[:], t8[:], -1.0, OP.add)
        ln1 = persist.tile([1, E], fp32)
        ln_series(ln1[:], d1, "lnA")
        elt = persist.tile([1, E], fp32)
        tt(elt[:], rppe_sb[:], ln1[:], OP.mult)
        el = persist.tile([1, 1], fp32)
        nc.vector.tensor_reduce(out=el[:], in_=elt[:], axis=AX.X, op=OP.add)

        st = persist.tile([1, 1], fp32)
        nc.vector.tensor_reduce(out=st[:], in_=ims_sb[:], axis=AX.X, op=OP.add)
        rst0 = persist.tile([1, 1], fp32)
        nc.vector.reciprocal(out=rst0[:], in_=st[:])
        at1 = persist.tile([1, 1], fp32)
        tt(at1[:], st[:], rst0[:], OP.mult)
        at2 = persist.tile([1, 1], fp32)
        ts(at2[:], at1[:], -1.0, OP.mult, 2.0, OP.add)
        rst = persist.tile([1, 1], fp32)
        tt(rst[:], rst0[:], at2[:], OP.mult)
        ippe = persist.tile([1, E], fp32)
        ts(ippe[:], ims_sb[:], rst[:], OP.mult)
        u8 = persist.tile([1, E], fp32)
        ts(u8[:], ippe[:], 8.0, OP.mult, 8.0e-9, OP.add)
        d2 = persist.tile([1, E], fp32)
        ts(d2[:], u8[:], -1.0, OP.add)
        ln2 = persist.tile([1, E], fp32)
        ln_series(ln2[:], d2, "lnB")
        lnip = persist.tile([1, E], fp32)
        ts(lnip[:], ln2[:], -LN8, OP.add)
        iet = persist.tile([1, E], fp32)
        tt(iet[:], ippe[:], lnip[:], OP.mult)
        ies = persist.tile([1, 1], fp32)
        nc.vector.tensor_reduce(out=ies[:], in_=iet[:], axis=AX.X, op=OP.add)
        # aux = el + (0.1/ln8) * sum(ippe*ln(ippe+eps))   [ies = -imp_entropy]
        sc = persist.tile([1, 1], fp32)
        ts(sc[:], ies[:], 0.1 / LN8, OP.mult)
        auxv = persist.tile([1, 1], fp32)
        tt(auxv[:], el[:], sc[:], OP.add)
        nc.sync.dma_start(out=aux_d[:, :], in_=auxv[:])

    nc.compile()
    return nc


def _get_nc():
    if "nc" not in _CACHE:
        _CACHE["nc"] = _build()
    return _CACHE["nc"]


def kernel(**inputs):
    global LAST_EXEC_NS, LAST_TRACE_DIR
    from concourse.bass_utils import run_bass_kernel_spmd

    inp = {k: np.ascontiguousarray(np.asarray(v), dtype=np.float32)
           for k, v in inputs.items()}
    x = inp["hidden_states"].reshape(N, H)

    nc = _get_nc()
    in_maps = []
    for c in range(NCORES):
        in_maps.append(dict(
            hidden_states=np.ascontiguousarray(x[c * TPC:(c + 1) * TPC]),
            wi1=inp["wi1"], bi1=inp["bi1"], wi2=inp["wi2"],
            bi2=inp["bi2"].reshape(1, 1),
            wr1=inp["wr1"], br1=inp["br1"], wr2=inp["wr2"],
            br2=inp["br2"].reshape(1, E),
            wu1=inp["wu1"], bu1=inp["bu1"], wu2=inp["wu2"],
            bu2=inp["bu2"].reshape(1, E),
            cmask=(np.arange(NCORES) < c).astype(np.float32).reshape(NCORES, 1),
        ))

    trace = bool(int(os.environ.get("KERNEL_TRACE", "0")))
    kwargs = {}
    if trace:
        _install_ntff_hook()
        import tempfile
        LAST_TRACE_DIR = tempfile.mkdtemp(prefix="adaptive_router_trace_")
        kwargs["tmpdir"] = LAST_TRACE_DIR
    res = run_bass_kernel_spmd(nc, in_maps, core_ids=list(range(NCORES)),
                               trace=trace, **kwargs)
    LAST_EXEC_NS = res.exec_time_ns

    disp = np.concatenate(
        [res.results[c]["dispatch"].reshape(TPC, E, CAP) for c in range(NCORES)], 0
    ).reshape(B, S, E, CAP)
    comb = np.concatenate(
        [res.results[c]["combine"].reshape(TPC, E, CAP) for c in range(NCORES)], 0
    ).reshape(B, S, E, CAP)
    probs = np.concatenate(
        [res.results[c]["probs"] for c in range(NCORES)], 0).reshape(B, S, E)
    impv = np.concatenate(
        [res.results[c]["importance"] for c in range(NCORES)], 0).reshape(B, S, 1)
    aux = np.float32(res.results[0]["aux"].reshape(()))
    return disp, comb, probs, aux, impv


# revision 16
# speedup vs baseline: 2.1685x; 1.1279x over previous
"""Trainium2 Bass kernel for nn_AdaptiveRouter (MoE dual-gate routing).

8 NeuronCores, data-parallel over tokens. Each core handles 512 tokens:
  - fp32 matmuls for importance MLP + both routers (decisions are
    numerically sensitive: mask threshold and top-k tie distances)
  - top-2 selection via exact fp32 compares on logits
  - capacity positions: strict-upper-triangular prefix matmuls within a
    128-token chunk, chunk carries, and an 8-core AllGather of per-core
    per-(k,expert) counts for the global k-major cumsum offsets
  - dispatch/combine [512,8,1536] f32 shards: zero-filled with large
    DMA stores, then one 1536-float row per (token,k) scattered via
    indirect DMA (OOB row index drops capacity-overflow entries)
  - second AllGather for aux-loss partial sums; ln() via series
"""

import math
import os
import sys
import types

sys.path.insert(0, "/opt/trn_rl_repo")

import numpy as np

# ---- problem constants (hardcoded; kernel.py must be self-contained) ----
B, S, H, E, TOPK, CF = 2, 2048, 1024, 8, 2, 1.5
N = B * S                      # 4096 tokens
NCORES = 8
TPC = N // NCORES              # 512 tokens per core
CAP = int(N * CF * TOPK / E)   # 1536
P = 128
NCH = TPC // P                 # 4 token chunks per core
H2 = H // 2
BIG = 8.0e6                    # element marker for dropped entries (OOB -> skipped)
LN8 = math.log(8.0)
NTERMS = 12                    # ln(1+x) series terms

LAST_EXEC_NS = None
LAST_TRACE_DIR = None

_CACHE = {}


def _install_ntff_hook():
    """Recreate antenv.axon_hooks (absent in this image) so
    run_bass_kernel_spmd(trace=True) can profile via libaxon_pjrt."""
    import antenv

    if "antenv.axon_hooks" not in sys.modules:
        mod = types.ModuleType("antenv.axon_hooks")
        mod._hook = None

        def set_axon_ntff_profile_hook(h):
            mod._hook = h

        def get_axon_ntff_profile_hook():
            return mod._hook

        mod.set_axon_ntff_profile_hook = set_axon_ntff_profile_hook
        mod.get_axon_ntff_profile_hook = get_axon_ntff_profile_hook
        sys.modules["antenv.axon_hooks"] = mod
        antenv.axon_hooks = mod
    mod = sys.modules["antenv.axon_hooks"]
    if mod._hook is None:
        from trn_agent_boot.trn_boot import _ntff_profile_via_ctypes

        mod.set_axon_ntff_profile_hook(
            _ntff_profile_via_ctypes("/opt/axon/libaxon_pjrt.so")
        )


def _build():
    import concourse.bacc as bacc
    import concourse.mybir as mybir
    import concourse.tile as tile
    import concourse.bass as bass
    from concourse.masks import make_identity, make_upper_triangular
    from concourse.tile import add_dep_helper
    from contextlib import ExitStack

    fp32 = mybir.dt.float32
    i32 = mybir.dt.int32
    AF = mybir.ActivationFunctionType
    OP = mybir.AluOpType
    AX = mybir.AxisListType

    nc = bacc.Bacc(None, target_bir_lowering=False)

    x_d = nc.declare_dram_parameter("hidden_states", [TPC, H], fp32, isOutput=False)
    wi1_d = nc.declare_dram_parameter("wi1", [H, H2], fp32, isOutput=False)
    bi1_d = nc.declare_dram_parameter("bi1", [H2], fp32, isOutput=False)
    wi2_d = nc.declare_dram_parameter("wi2", [H2, 1], fp32, isOutput=False)
    bi2_d = nc.declare_dram_parameter("bi2", [1, 1], fp32, isOutput=False)
    wr1_d = nc.declare_dram_parameter("wr1", [H, H], fp32, isOutput=False)
    br1_d = nc.declare_dram_parameter("br1", [H], fp32, isOutput=False)
    wr2_d = nc.declare_dram_parameter("wr2", [H, E], fp32, isOutput=False)
    br2_d = nc.declare_dram_parameter("br2", [1, E], fp32, isOutput=False)
    wu1_d = nc.declare_dram_parameter("wu1", [H, H], fp32, isOutput=False)
    bu1_d = nc.declare_dram_parameter("bu1", [H], fp32, isOutput=False)
    wu2_d = nc.declare_dram_parameter("wu2", [H, E], fp32, isOutput=False)
    bu2_d = nc.declare_dram_parameter("bu2", [1, E], fp32, isOutput=False)
    cm_d = nc.declare_dram_parameter("cmask", [NCORES, 1], fp32, isOutput=False)

    disp_d = nc.declare_dram_parameter("dispatch", [TPC * E, CAP], fp32, isOutput=True)
    comb_d = nc.declare_dram_parameter("combine", [TPC * E, CAP], fp32, isOutput=True)
    prob_d = nc.declare_dram_parameter("probs", [TPC, E], fp32, isOutput=True)
    imp_d = nc.declare_dram_parameter("importance", [TPC, 1], fp32, isOutput=True)
    aux_d = nc.declare_dram_parameter("aux", [1, 1], fp32, isOutput=True)

    IOA = bass.IndirectOffsetOnAxis
    RG = [list(range(NCORES))]

    with tile.TileContext(nc) as tc, ExitStack() as ctx:
        const = ctx.enter_context(tc.tile_pool(name="const", bufs=1))
        wpool = ctx.enter_context(tc.tile_pool(name="wpool", bufs=1))
        persist = ctx.enter_context(tc.tile_pool(name="persist", bufs=1))
        xraw_p = ctx.enter_context(tc.tile_pool(name="xraw", bufs=2))
        sm = ctx.enter_context(tc.tile_pool(name="sm", bufs=3))       # small transients
        l1ps = ctx.enter_context(tc.tile_pool(name="l1ps", bufs=2, space="PSUM"))
        l2ps = ctx.enter_context(tc.tile_pool(name="l2ps", bufs=2, space="PSUM"))
        ccps_p = ctx.enter_context(tc.tile_pool(name="ccps", bufs=2, space="PSUM"))
        dram = ctx.enter_context(tc.tile_pool(name="dram", bufs=1, space="DRAM"))

        def ts(out, in0, s1, op0, s2=None, op1=None, eng=None):
            e = eng if eng is not None else nc.vector
            if s2 is None:
                e.tensor_scalar(out=out, in0=in0, scalar1=s1, scalar2=None, op0=op0)
            else:
                e.tensor_scalar(out=out, in0=in0, scalar1=s1, scalar2=s2,
                                op0=op0, op1=op1)

        def tt(out, a, b, op):
            nc.vector.tensor_tensor(out=out, in0=a, in1=b, op=op)

        # ---------------- constants ----------------
        ident = const.tile([P, P], fp32)
        make_identity(nc, ident[:])
        ut = const.tile([P, P], fp32)
        make_upper_triangular(nc, ut[:], val=1.0, diag=False)  # strictly upper
        iota8 = const.tile([P, E], fp32)
        nc.gpsimd.iota(iota8[:], pattern=[[1, E]], base=0, channel_multiplier=0,
                       allow_small_or_imprecise_dtypes=True)
        iota_col8 = const.tile([P, 1], fp32)
        nc.gpsimd.iota(iota_col8[:], pattern=[[0, 1]], base=0, channel_multiplier=E,
                       allow_small_or_imprecise_dtypes=True)  # p*8
        iota_tok = const.tile([P, 1], fp32)
        nc.gpsimd.iota(iota_tok[:], pattern=[[0, 1]], base=0,
                       channel_multiplier=E * CAP,
                       allow_small_or_imprecise_dtypes=True)  # p*12288
        ones_row = const.tile([1, P], fp32)
        nc.vector.memset(ones_row[:], 1.0)
        ones_col = const.tile([P, 1], fp32)
        nc.vector.memset(ones_col[:], 1.0)
        ones8 = const.tile([NCORES, 1], fp32)
        nc.vector.memset(ones8[:], 1.0)
        zero_sb = const.tile([P, 2048], fp32)
        nc.vector.memset(zero_sb[:], 0.0)

        # ---------------- x load + transpose (PE critical path first) -----
        xT = wpool.tile([P, 8, TPC], fp32)
        for tcch in range(NCH):
            x_raw = xraw_p.tile([P, H], fp32, tag="xraw")
            nc.sync.dma_start(out=x_raw[:], in_=x_d[tcch * P:(tcch + 1) * P, :])
            for kc in range(8):
                tps = l2ps.tile([P, P], fp32, tag="l2")
                nc.tensor.transpose(out=tps[:], in_=x_raw[:, kc * P:(kc + 1) * P],
                                    identity=ident[:])
                nc.vector.tensor_copy(out=xT[:, kc, tcch * P:(tcch + 1) * P], in_=tps[:])

        # ---------------- weight loads ----------------
        wi1_sb = wpool.tile([P, 8, H2], fp32)
        nc.sync.dma_start(out=wi1_sb[:], in_=wi1_d[:, :].rearrange("(k p) m -> p k m", p=P))
        wr1_sb = wpool.tile([P, 8, H], fp32)
        nc.sync.dma_start(out=wr1_sb[:], in_=wr1_d[:, :].rearrange("(k p) m -> p k m", p=P))
        wu1_sb = wpool.tile([P, 8, H], fp32)
        nc.sync.dma_start(out=wu1_sb[:], in_=wu1_d[:, :].rearrange("(k p) m -> p k m", p=P))
        wi2_sb = wpool.tile([P, 4, 1], fp32)
        nc.sync.dma_start(out=wi2_sb[:], in_=wi2_d[:, :].rearrange("(k p) m -> p k m", p=P))
        wr2_sb = wpool.tile([P, 8, E], fp32)
        nc.sync.dma_start(out=wr2_sb[:], in_=wr2_d[:, :].rearrange("(k p) m -> p k m", p=P))
        wu2_sb = wpool.tile([P, 8, E], fp32)
        nc.sync.dma_start(out=wu2_sb[:], in_=wu2_d[:, :].rearrange("(k p) m -> p k m", p=P))
        bi1c = wpool.tile([P, 4], fp32)
        nc.sync.dma_start(out=bi1c[:], in_=bi1_d[:].rearrange("(m p) -> p m", p=P))
        br1c = wpool.tile([P, 8], fp32)
        nc.sync.dma_start(out=br1c[:], in_=br1_d[:].rearrange("(m p) -> p m", p=P))
        bu1c = wpool.tile([P, 8], fp32)
        nc.sync.dma_start(out=bu1c[:], in_=bu1_d[:].rearrange("(m p) -> p m", p=P))
        br2row = wpool.tile([1, E], fp32)
        nc.sync.dma_start(out=br2row[:], in_=br2_d[:, :])
        bu2row = wpool.tile([1, E], fp32)
        nc.sync.dma_start(out=bu2row[:], in_=bu2_d[:, :])
        bi2row = wpool.tile([1, 1], fp32)
        nc.sync.dma_start(out=bi2row[:], in_=bi2_d[:, :])
        cmask_sb = wpool.tile([NCORES, 1], fp32)
        nc.sync.dma_start(out=cmask_sb[:], in_=cm_d[:, :])

        # ---------------- zero-fill dispatch/combine (fills DMA idle) -----
        zfill_insts = {"dispatch": [], "combine": []}
        for name, dd in (("dispatch", disp_d), ("combine", comb_d)):
            flat = dd[:, :].rearrange("r c -> (r c)")
            chunk = P * 2048
            for i in range((TPC * E * CAP) // chunk):   # 24 x 1MiB each
                ins = nc.sync.dma_start(
                    out=flat[i * chunk:(i + 1) * chunk].rearrange("(p f) -> p f", p=P),
                    in_=zero_sb[:],
                )
                zfill_insts[name].append(ins)

        # ---------------- persistent small tensors ----------------
        mask_sb = persist.tile([P, NCH], fp32)
        li_sb = persist.tile([P, NCH, E], fp32)
        logits_sb = persist.tile([P, NCH, E], fp32)
        rmax_sb = persist.tile([P, NCH], fp32)
        probs_sb = persist.tile([P, NCH, E], fp32)
        ohs = persist.tile([P, NCH, 2, E], fp32)
        idx_sb = persist.tile([P, NCH, 2], fp32)
        w_sb = persist.tile([P, NCH, 2], fp32)
        prefix_sb = persist.tile([P, NCH, 2, E], fp32)  # within-chunk excl prefix
        chcnt = persist.tile([1, 2, NCH, E], fp32)   # per (k, chunk) counts
        carr = persist.tile([1, 2, NCH, E], fp32)    # per (k, chunk) carry
        probsum_sb = persist.tile([1, E], fp32)
        impsum_sb = persist.tile([1, E], fp32)
        agin = persist.tile([1, 2 * E], fp32)        # counts AG payload
        agin2 = persist.tile([1, 2 * E], fp32)       # aux AG payload
        cnt_all = persist.tile([NCORES, 2 * E], fp32)
        aux_all = persist.tile([NCORES, 2 * E], fp32)
        off0 = persist.tile([1, E], fp32)
        off1 = persist.tile([1, E], fp32)
        tot0 = persist.tile([1, E], fp32)
        addv = persist.tile([1, NCH * 2 * E], fp32)  # carry+offset per (tc,k)
        rppe_sb = persist.tile([1, E], fp32)
        ims_sb = persist.tile([1, E], fp32)

        # ---------------- layer 1 helper ----------------
        def layer1(w1_sb, nmc, bias_col):
            hb = wpool.tile([P, 8, TPC], fp32, tag="hbuf", name="hbuf")
            for mc in range(nmc):
                ps = l1ps.tile([P, TPC], fp32, tag="l1")
                for kc in range(8):
                    nc.tensor.matmul(out=ps[:], lhsT=w1_sb[:, kc, mc * P:(mc + 1) * P],
                                     rhs=xT[:, kc, :], start=(kc == 0), stop=(kc == 7))
                nc.scalar.activation(out=hb[:, mc, :], in_=ps[:], func=AF.Relu,
                                     bias=bias_col[:, mc:mc + 1], scale=1.0)
            return hb

        # --- importance net ---
        hbuf = layer1(wi1_sb, 4, bi1c)
        for tcch in range(NCH):
            zps = l2ps.tile([P, 1], fp32, tag="l2")
            for kc in range(4):
                nc.tensor.matmul(out=zps[:], lhsT=hbuf[:, kc, tcch * P:(tcch + 1) * P],
                                 rhs=wi2_sb[:, kc, :], start=(kc == 0), stop=False)
            nc.tensor.matmul(out=zps[:], lhsT=ones_row[:, :], rhs=bi2row[:, :],
                             start=False, stop=True)
            ts(mask_sb[:, tcch:tcch + 1], zps[:], 0.0, OP.is_gt)
            impv = sm.tile([P, 1], fp32, tag="impv")
            nc.scalar.activation(out=impv[:], in_=zps[:], func=AF.Sigmoid)
            nc.sync.dma_start(out=imp_d[tcch * P:(tcch + 1) * P, :], in_=impv[:])

        # --- router_important ---
        hbuf = layer1(wr1_sb, 8, br1c)
        for tcch in range(NCH):
            lps = l2ps.tile([P, E], fp32, tag="l2")
            for kc in range(8):
                nc.tensor.matmul(out=lps[:], lhsT=hbuf[:, kc, tcch * P:(tcch + 1) * P],
                                 rhs=wr2_sb[:, kc, :], start=(kc == 0), stop=False)
            nc.tensor.matmul(out=lps[:], lhsT=ones_row[:, :], rhs=br2row[:, :],
                             start=False, stop=True)
            nc.vector.tensor_copy(out=li_sb[:, tcch, :], in_=lps[:])

        # --- router_unimportant + decision chain (everything the counts
        #     AllGather needs: logits -> top2 -> one-hots -> counts) ---
        hbuf = layer1(wu1_sb, 8, bu1c)
        for tcch in range(NCH):
            tsl = slice(tcch * P, (tcch + 1) * P)
            lups = l2ps.tile([P, E], fp32, tag="l2")
            for kc in range(8):
                nc.tensor.matmul(out=lups[:], lhsT=hbuf[:, kc, tsl],
                                 rhs=wu2_sb[:, kc, :], start=(kc == 0), stop=False)
            nc.tensor.matmul(out=lups[:], lhsT=ones_row[:, :], rhs=bu2row[:, :],
                             start=False, stop=True)

            m = mask_sb[:, tcch:tcch + 1]
            invm = sm.tile([P, 1], fp32, tag="invm")
            ts(invm[:], m, -1.0, OP.mult, 1.0, OP.add)
            t1 = sm.tile([P, E], fp32, tag="t1")
            ts(t1[:], li_sb[:, tcch, :], m, OP.mult)
            t2 = sm.tile([P, E], fp32, tag="t2")
            ts(t2[:], lups[:], invm[:], OP.mult)
            tt(logits_sb[:, tcch, :], t1[:], t2[:], OP.add)
            logits = logits_sb[:, tcch, :]

            # top-2 on logits (exact fp32)
            rmax = rmax_sb[:, tcch:tcch + 1]
            nc.vector.tensor_reduce(out=rmax, in_=logits, axis=AX.X, op=OP.max)
            eq1 = sm.tile([P, E], fp32, tag="eq1")
            ts(eq1[:], logits, rmax, OP.is_equal)
            b1 = sm.tile([P, E], fp32, tag="b1")
            ts(b1[:], eq1[:], -999.0, OP.mult, 999.0, OP.add)
            c1 = sm.tile([P, E], fp32, tag="c1")
            tt(c1[:], b1[:], iota8[:], OP.add)
            nc.vector.tensor_reduce(out=idx_sb[:, tcch, 0:1], in_=c1[:], axis=AX.X,
                                    op=OP.min)
            ts(ohs[:, tcch, 0, :], iota8[:], idx_sb[:, tcch, 0:1], OP.is_equal)
            negm = sm.tile([P, E], fp32, tag="negm")
            ts(negm[:], ohs[:, tcch, 0, :], -1.0e9, OP.mult)
            lm = sm.tile([P, E], fp32, tag="lm")
            tt(lm[:], logits, negm[:], OP.add)
            l2v = sm.tile([P, 1], fp32, tag="l2v")
            nc.vector.tensor_reduce(out=l2v[:], in_=lm[:], axis=AX.X, op=OP.max)
            eq2 = sm.tile([P, E], fp32, tag="eq2")
            ts(eq2[:], lm[:], l2v[:], OP.is_equal)
            b2 = sm.tile([P, E], fp32, tag="b2")
            ts(b2[:], eq2[:], -999.0, OP.mult, 999.0, OP.add)
            c2 = sm.tile([P, E], fp32, tag="c2")
            tt(c2[:], b2[:], iota8[:], OP.add)
            nc.vector.tensor_reduce(out=idx_sb[:, tcch, 1:2], in_=c2[:], axis=AX.X,
                                    op=OP.min)
            ts(ohs[:, tcch, 1, :], iota8[:], idx_sb[:, tcch, 1:2], OP.is_equal)

            for k in range(2):
                ccp = ccps_p.tile([1, E], fp32, tag="ccp")
                nc.tensor.matmul(out=ccp[:], lhsT=ones_col[:, :],
                                 rhs=ohs[:, tcch, k, :], start=True, stop=True)
                nc.vector.tensor_copy(out=chcnt[:, k, tcch, :], in_=ccp[:])

        # totals + carries, then fire the counts AllGather ASAP
        for k in range(2):
            nc.vector.tensor_copy(out=carr[:, k, 1, :], in_=chcnt[:, k, 0, :])
            tt(carr[:, k, 2, :], carr[:, k, 1, :], chcnt[:, k, 1, :], OP.add)
            tt(carr[:, k, 3, :], carr[:, k, 2, :], chcnt[:, k, 2, :], OP.add)
            tt(agin[:, k * E:(k + 1) * E], carr[:, k, 3, :], chcnt[:, k, 3, :], OP.add)

        ag_in_d = dram.tile([1, 2 * E], fp32)
        ag_out_d = dram.tile([NCORES, 2 * E], fp32, addr_space="Shared")
        nc.gpsimd.dma_start(out=ag_in_d[:], in_=agin[:])
        nc.gpsimd.collective_compute(
            "AllGather", mybir.AluOpType.bypass,
            ins=[ag_in_d[:]], outs=[ag_out_d[:]], replica_groups=RG,
        )
        nc.gpsimd.dma_start(out=cnt_all[:], in_=ag_out_d[:])

        # --- value chain per chunk (overlaps the AllGather): softmax,
        #     gate weights, masked-prob sums, prefix matmuls ---
        for tcch in range(NCH):
            tsl = slice(tcch * P, (tcch + 1) * P)
            logits = logits_sb[:, tcch, :]
            nrmax = sm.tile([P, 1], fp32, tag="nrmax")
            ts(nrmax[:], rmax_sb[:, tcch:tcch + 1], -1.0, OP.mult)
            exp_t = sm.tile([P, E], fp32, tag="exp_t")
            ssum = sm.tile([P, 1], fp32, tag="ssum")
            nc.scalar.activation(out=exp_t[:], in_=logits, func=AF.Exp,
                                 bias=nrmax[:], scale=1.0, accum_out=ssum[:])
            rs0 = sm.tile([P, 1], fp32, tag="rs0")
            nc.vector.reciprocal(out=rs0[:], in_=ssum[:])
            nt1 = sm.tile([P, 1], fp32, tag="nt1")
            tt(nt1[:], ssum[:], rs0[:], OP.mult)
            nt2 = sm.tile([P, 1], fp32, tag="nt2")
            ts(nt2[:], nt1[:], -1.0, OP.mult, 2.0, OP.add)
            rs = sm.tile([P, 1], fp32, tag="rs")
            tt(rs[:], rs0[:], nt2[:], OP.mult)
            ts(probs_sb[:, tcch, :], exp_t[:], rs[:], OP.mult)
            nc.sync.dma_start(out=prob_d[tsl, :], in_=probs_sb[:, tcch, :])

            scr1 = sm.tile([P, E], fp32, tag="scr1")
            p1 = sm.tile([P, 1], fp32, tag="p1")
            tt(scr1[:], probs_sb[:, tcch, :], ohs[:, tcch, 0, :], OP.mult)
            nc.vector.tensor_reduce(out=p1[:], in_=scr1[:], axis=AX.X, op=OP.add)
            scr2 = sm.tile([P, E], fp32, tag="scr2")
            p2 = sm.tile([P, 1], fp32, tag="p2")
            tt(scr2[:], probs_sb[:, tcch, :], ohs[:, tcch, 1, :], OP.mult)
            nc.vector.tensor_reduce(out=p2[:], in_=scr2[:], axis=AX.X, op=OP.add)
            ws = sm.tile([P, 1], fp32, tag="ws")
            tt(ws[:], p1[:], p2[:], OP.add)
            rw0 = sm.tile([P, 1], fp32, tag="rw0")
            nc.vector.reciprocal(out=rw0[:], in_=ws[:])
            wt1 = sm.tile([P, 1], fp32, tag="wt1")
            tt(wt1[:], ws[:], rw0[:], OP.mult)
            wt2 = sm.tile([P, 1], fp32, tag="wt2")
            ts(wt2[:], wt1[:], -1.0, OP.mult, 2.0, OP.add)
            rw = sm.tile([P, 1], fp32, tag="rw")
            tt(rw[:], rw0[:], wt2[:], OP.mult)
            tt(w_sb[:, tcch, 0:1], p1[:], rw[:], OP.mult)
            tt(w_sb[:, tcch, 1:2], p2[:], rw[:], OP.mult)

            pm = sm.tile([P, E], fp32, tag="pm")
            ts(pm[:], probs_sb[:, tcch, :], mask_sb[:, tcch:tcch + 1], OP.mult)
            pst = ccps_p.tile([1, E], fp32, tag="ccp")
            nc.tensor.matmul(out=pst[:], lhsT=ones_col[:, :],
                             rhs=probs_sb[:, tcch, :], start=True, stop=True)
            if tcch == 0:
                nc.vector.tensor_copy(out=probsum_sb[:], in_=pst[:])
            else:
                tt(probsum_sb[:], probsum_sb[:], pst[:], OP.add)
            ist = ccps_p.tile([1, E], fp32, tag="ccp")
            nc.tensor.matmul(out=ist[:], lhsT=ones_col[:, :],
                             rhs=pm[:], start=True, stop=True)
            if tcch == 0:
                nc.vector.tensor_copy(out=impsum_sb[:], in_=ist[:])
            else:
                tt(impsum_sb[:], impsum_sb[:], ist[:], OP.add)

            for k in range(2):
                pfx = l2ps.tile([P, E], fp32, tag="l2")
                nc.tensor.matmul(out=pfx[:], lhsT=ut[:, :],
                                 rhs=ohs[:, tcch, k, :], start=True, stop=True)
                nc.vector.tensor_copy(out=prefix_sb[:, tcch, k, :], in_=pfx[:])

        # second AllGather for the aux partial sums (overlaps scatter tail)
        nc.vector.tensor_copy(out=agin2[:, 0:E], in_=probsum_sb[:])
        nc.vector.tensor_copy(out=agin2[:, E:2 * E], in_=impsum_sb[:])
        ag2_in_d = dram.tile([1, 2 * E], fp32)
        ag2_out_d = dram.tile([NCORES, 2 * E], fp32, addr_space="Shared")
        nc.gpsimd.dma_start(out=ag2_in_d[:], in_=agin2[:])
        nc.gpsimd.collective_compute(
            "AllGather", mybir.AluOpType.bypass,
            ins=[ag2_in_d[:]], outs=[ag2_out_d[:]], replica_groups=RG,
        )
        nc.gpsimd.dma_start(out=aux_all[:], in_=ag2_out_d[:])

        # cross-core offsets from the counts AG
        ms0 = ccps_p.tile([1, E], fp32, tag="ccp")
        nc.tensor.matmul(out=ms0[:], lhsT=cmask_sb[:, :], rhs=cnt_all[:, 0:E],
                         start=True, stop=True)
        nc.vector.tensor_copy(out=off0[:], in_=ms0[:])
        t0p = ccps_p.tile([1, E], fp32, tag="ccp")
        nc.tensor.matmul(out=t0p[:], lhsT=ones8[:, :], rhs=cnt_all[:, 0:E],
                         start=True, stop=True)
        nc.vector.tensor_copy(out=tot0[:], in_=t0p[:])
        ms1 = ccps_p.tile([1, E], fp32, tag="ccp")
        nc.tensor.matmul(out=ms1[:], lhsT=cmask_sb[:, :], rhs=cnt_all[:, E:2 * E],
                         start=True, stop=True)
        tt(off1[:], ms1[:], tot0[:], OP.add)

        for tcch in range(NCH):
            for k in range(2):
                sl = (tcch * 2 + k) * E
                offk = off0 if k == 0 else off1
                if tcch == 0:
                    nc.vector.tensor_copy(out=addv[:, sl:sl + E], in_=offk[:])
                else:
                    tt(addv[:, sl:sl + E], carr[:, k, tcch, :], offk[:], OP.add)
        bc_ps = l2ps.tile([P, NCH * 2 * E], fp32, tag="l2")
        nc.tensor.matmul(out=bc_ps[:], lhsT=ones_row[:, :], rhs=addv[:, :],
                         start=True, stop=True)

        # final positions + single-element scatter: flat index
        # (t*E + e)*CAP + pos; dispatch writes 1.0, combine writes w.
        disp_flat = disp_d[:, :].rearrange("r c -> (r c)")[:, None]
        comb_flat = comb_d[:, :].rearrange("r c -> (r c)")[:, None]
        for tcch in range(NCH):
            for k in range(2):
                sl = (tcch * 2 + k) * E
                padd = sm.tile([P, E], fp32, tag="padd")
                tt(padd[:], prefix_sb[:, tcch, k, :], bc_ps[:, sl:sl + E], OP.add)
                scr = sm.tile([P, E], fp32, tag="scr")
                posk = sm.tile([P, 1], fp32, tag="posk")
                tt(scr[:], padd[:], ohs[:, tcch, k, :], OP.mult)
                nc.vector.tensor_reduce(out=posk[:], in_=scr[:], axis=AX.X, op=OP.add)
                keep = sm.tile([P, 1], fp32, tag="keep")
                ts(keep[:], posk[:], float(CAP), OP.is_lt)
                posc = sm.tile([P, 1], fp32, tag="posc")
                ts(posc[:], posk[:], float(CAP - 1), OP.min)
                r0 = sm.tile([P, 1], fp32, tag="r0")
                ts(r0[:], idx_sb[:, tcch, k:k + 1], float(CAP), OP.mult,
                   float(tcch * P * E * CAP), OP.add)
                r1 = sm.tile([P, 1], fp32, tag="r1")
                tt(r1[:], r0[:], iota_tok[:], OP.add)
                r2 = sm.tile([P, 1], fp32, tag="r2")
                tt(r2[:], r1[:], posc[:], OP.add)
                nk = sm.tile([P, 1], fp32, tag="nk")
                ts(nk[:], keep[:], -BIG, OP.mult, BIG, OP.add)
                rf = sm.tile([P, 1], fp32, tag="rf")
                tt(rf[:], r2[:], nk[:], OP.add)
                ri = sm.tile([P, 1], i32, tag="ri")
                nc.vector.tensor_copy(out=ri[:], in_=rf[:])

                s1 = nc.gpsimd.indirect_dma_start(
                    out=disp_flat, out_offset=IOA(ap=ri[:, 0:1], axis=0),
                    in_=ones_col[:, 0:1], in_offset=None,
                    bounds_check=TPC * E * CAP - 1, oob_is_err=False)
                s2 = nc.gpsimd.indirect_dma_start(
                    out=comb_flat, out_offset=IOA(ap=ri[:, 0:1], axis=0),
                    in_=w_sb[:, tcch, k:k + 1], in_offset=None,
                    bounds_check=TPC * E * CAP - 1, oob_is_err=False)
                for z in zfill_insts["dispatch"]:
                    add_dep_helper(s1.ins, z.ins, reason="scatter after zero-fill")
                for z in zfill_insts["combine"]:
                    add_dep_helper(s2.ins, z.ins, reason="scatter after zero-fill")

        # ---------------- aux loss (from the aux AllGather) ----------------
        def ln_series(out_sb, d_sb, tagp):
            # ln(1+d) = d*(1 - d*(1/2 - d*(1/3 - ...)))
            s = persist.tile([1, E], fp32, name=f"{tagp}_s")
            nc.vector.memset(s[:], 1.0 / NTERMS)
            for i in range(NTERMS - 1, 0, -1):
                mtmp = sm.tile([1, E], fp32, tag=f"{tagp}_m")
                tt(mtmp[:], d_sb[:], s[:], OP.mult)
                ts(s[:], mtmp[:], -1.0, OP.mult, 1.0 / i, OP.add)
            tt(out_sb, d_sb[:], s[:], OP.mult)

        rpp = ccps_p.tile([1, E], fp32, tag="ccp")
        nc.tensor.matmul(out=rpp[:], lhsT=ones8[:, :], rhs=aux_all[:, 0:E],
                         start=True, stop=True)
        ts(rppe_sb[:], rpp[:], 1.0 / N, OP.mult)
        imt = ccps_p.tile([1, E], fp32, tag="ccp")
        nc.tensor.matmul(out=imt[:], lhsT=ones8[:, :], rhs=aux_all[:, E:2 * E],
                         start=True, stop=True)
        ts(ims_sb[:], imt[:], 1.0e-9, OP.add)

        t8 = persist.tile([1, E], fp32)
        ts(t8[:], rppe_sb[:], 8.0, OP.mult, 1.0e-9, OP.add)
        d1 = persist.tile([1, E], fp32)
        ts(d1[:], t8[:], -1.0, OP.add)
        ln1 = persist.tile([1, E], fp32)
        ln_series(ln1[:], d1, "lnA")
        elt = persist.tile([1, E], fp32)
        tt(elt[:], rppe_sb[:], ln1[:], OP.mult)
        el = persist.tile([1, 1], fp32)
        nc.vector.tensor_reduce(out=el[:], in_=elt[:], axis=AX.X, op=OP.add)

        st = persist.tile([1, 1], fp32)
        nc.vector.tensor_reduce(out=st[:], in_=ims_sb[:], axis=AX.X, op=OP.add)
        rst0 = persist.tile([1, 1], fp32)
        nc.vector.reciprocal(out=rst0[:], in_=st[:])
        at1 = persist.tile([1, 1], fp32)
        tt(at1[:], st[:], rst0[:], OP.mult)
        at2 = persist.tile([1, 1], fp32)
        ts(at2[:], at1[:], -1.0, OP.mult, 2.0, OP.add)
        rst = persist.tile([1, 1], fp32)
        tt(rst[:], rst0[:], at2[:], OP.mult)
        ippe = persist.tile([1, E], fp32)
        ts(ippe[:], ims_sb[:], rst[:], OP.mult)
        u8 = persist.tile([1, E], fp32)
        ts(u8[:], ippe[:], 8.0, OP.mult, 8.0e-9, OP.add)
        d2 = persist.tile([1, E], fp32)
        ts(d2[:], u8[:], -1.0, OP.add)
        ln2 = persist.tile([1, E], fp32)
        ln_series(ln2[:], d2, "lnB")
        lnip = persist.tile([1, E], fp32)
        ts(lnip[:], ln2[:], -LN8, OP.add)
        iet = persist.tile([1, E], fp32)
        tt(iet[:], ippe[:], lnip[:], OP.mult)
        ies = persist.tile([1, 1], fp32)
        nc.vector.tensor_reduce(out=ies[:], in_=iet[:], axis=AX.X, op=OP.add)
        # aux = el + (0.1/ln8) * sum(ippe*ln(ippe+eps))   [ies = -imp_entropy]
        sc = persist.tile([1, 1], fp32)
        ts(sc[:], ies[:], 0.1 / LN8, OP.mult)
        auxv = persist.tile([1, 1], fp32)
        tt(auxv[:], el[:], sc[:], OP.add)
        nc.sync.dma_start(out=aux_d[:, :], in_=auxv[:])

    nc.compile()
    return nc


def _get_nc():
    if "nc" not in _CACHE:
        _CACHE["nc"] = _build()
    return _CACHE["nc"]


def kernel(**inputs):
    global LAST_EXEC_NS, LAST_TRACE_DIR
    from concourse.bass_utils import run_bass_kernel_spmd

    inp = {k: np.ascontiguousarray(np.asarray(v), dtype=np.float32)
           for k, v in inputs.items()}
    x = inp["hidden_states"].reshape(N, H)

    nc = _get_nc()
    in_maps = []
    for c in range(NCORES):
        in_maps.append(dict(
            hidden_states=np.ascontiguousarray(x[c * TPC:(c + 1) * TPC]),
            wi1=inp["wi1"], bi1=inp["bi1"], wi2=inp["wi2"],
            bi2=inp["bi2"].reshape(1, 1),
            wr1=inp["wr1"], br1=inp["br1"], wr2=inp["wr2"],
            br2=inp["br2"].reshape(1, E),
            wu1=inp["wu1"], bu1=inp["bu1"], wu2=inp["wu2"],
            bu2=inp["bu2"].reshape(1, E),
            cmask=(np.arange(NCORES) < c).astype(np.float32).reshape(NCORES, 1),
        ))

    trace = bool(int(os.environ.get("KERNEL_TRACE", "0")))
    kwargs = {}
    if trace:
        _install_ntff_hook()
        import tempfile
        LAST_TRACE_DIR = tempfile.mkdtemp(prefix="adaptive_router_trace_")
        kwargs["tmpdir"] = LAST_TRACE_DIR
    res = run_bass_kernel_spmd(nc, in_maps, core_ids=list(range(NCORES)),
                               trace=trace, **kwargs)
    LAST_EXEC_NS = res.exec_time_ns

    disp = np.concatenate(
        [res.results[c]["dispatch"].reshape(TPC, E, CAP) for c in range(NCORES)], 0
    ).reshape(B, S, E, CAP)
    comb = np.concatenate(
        [res.results[c]["combine"].reshape(TPC, E, CAP) for c in range(NCORES)], 0
    ).reshape(B, S, E, CAP)
    probs = np.concatenate(
        [res.results[c]["probs"] for c in range(NCORES)], 0).reshape(B, S, E)
    impv = np.concatenate(
        [res.results[c]["importance"] for c in range(NCORES)], 0).reshape(B, S, 1)
    aux = np.float32(res.results[0]["aux"].reshape(()))
    return disp, comb, probs, aux, impv


# revision 20
# speedup vs baseline: 2.2121x; 1.0201x over previous
"""Trainium2 Bass kernel for nn_AdaptiveRouter (MoE dual-gate routing).

8 NeuronCores, data-parallel over tokens. Each core handles 512 tokens:
  - fp32 matmuls for importance MLP + both routers (decisions are
    numerically sensitive: mask threshold and top-k tie distances)
  - top-2 selection via exact fp32 compares on logits
  - capacity positions: strict-upper-triangular prefix matmuls within a
    128-token chunk, chunk carries, and an 8-core AllGather of per-core
    per-(k,expert) counts for the global k-major cumsum offsets
  - dispatch/combine [512,8,1536] f32 shards: zero-filled with large
    DMA stores, then one 1536-float row per (token,k) scattered via
    indirect DMA (OOB row index drops capacity-overflow entries)
  - second AllGather for aux-loss partial sums; ln() via series
"""

import math
import os
import sys
import types

sys.path.insert(0, "/opt/trn_rl_repo")

import numpy as np

# ---- problem constants (hardcoded; kernel.py must be self-contained) ----
B, S, H, E, TOPK, CF = 2, 2048, 1024, 8, 2, 1.5
N = B * S                      # 4096 tokens
NCORES = 8
TPC = N // NCORES              # 512 tokens per core
CAP = int(N * CF * TOPK / E)   # 1536
P = 128
NCH = TPC // P                 # 4 token chunks per core
H2 = H // 2
BIG = 8.0e6                    # element marker for dropped entries (OOB -> skipped)
LN8 = math.log(8.0)
NTERMS = 12                    # ln(1+x) series terms

LAST_EXEC_NS = None
LAST_TRACE_DIR = None

_CACHE = {}


def _install_ntff_hook():
    """Recreate antenv.axon_hooks (absent in this image) so
    run_bass_kernel_spmd(trace=True) can profile via libaxon_pjrt."""
    import antenv

    if "antenv.axon_hooks" not in sys.modules:
        mod = types.ModuleType("antenv.axon_hooks")
        mod._hook = None

        def set_axon_ntff_profile_hook(h):
            mod._hook = h

        def get_axon_ntff_profile_hook():
            return mod._hook

        mod.set_axon_ntff_profile_hook = set_axon_ntff_profile_hook
        mod.get_axon_ntff_profile_hook = get_axon_ntff_profile_hook
        sys.modules["antenv.axon_hooks"] = mod
        antenv.axon_hooks = mod
    mod = sys.modules["antenv.axon_hooks"]
    if mod._hook is None:
        from trn_agent_boot.trn_boot import _ntff_profile_via_ctypes

        mod.set_axon_ntff_profile_hook(
            _ntff_profile_via_ctypes("/opt/axon/libaxon_pjrt.so")
        )


def _build():
    import concourse.bacc as bacc
    import concourse.mybir as mybir
    import concourse.tile as tile
    import concourse.bass as bass
    from concourse.tile import add_dep_helper
    from contextlib import ExitStack

    fp32 = mybir.dt.float32
    i32 = mybir.dt.int32
    AF = mybir.ActivationFunctionType
    OP = mybir.AluOpType
    AX = mybir.AxisListType

    nc = bacc.Bacc(None, target_bir_lowering=False)

    x_d = nc.declare_dram_parameter("hidden_states", [TPC, H], fp32, isOutput=False)
    wi1_d = nc.declare_dram_parameter("wi1", [H, H2], fp32, isOutput=False)
    bi1_d = nc.declare_dram_parameter("bi1", [H2], fp32, isOutput=False)
    wi2_d = nc.declare_dram_parameter("wi2", [H2, 1], fp32, isOutput=False)
    bi2_d = nc.declare_dram_parameter("bi2", [1, 1], fp32, isOutput=False)
    wr1_d = nc.declare_dram_parameter("wr1", [H, H], fp32, isOutput=False)
    br1_d = nc.declare_dram_parameter("br1", [H], fp32, isOutput=False)
    wr2_d = nc.declare_dram_parameter("wr2", [H, E], fp32, isOutput=False)
    br2_d = nc.declare_dram_parameter("br2", [1, E], fp32, isOutput=False)
    wu1_d = nc.declare_dram_parameter("wu1", [H, H], fp32, isOutput=False)
    bu1_d = nc.declare_dram_parameter("bu1", [H], fp32, isOutput=False)
    wu2_d = nc.declare_dram_parameter("wu2", [H, E], fp32, isOutput=False)
    bu2_d = nc.declare_dram_parameter("bu2", [1, E], fp32, isOutput=False)
    cm_d = nc.declare_dram_parameter("cmask", [NCORES, 1], fp32, isOutput=False)
    cc_d = nc.declare_dram_parameter("cconst", [P, 266], fp32, isOutput=False)
    cr_d = nc.declare_dram_parameter("crow", [1, TPC], fp32, isOutput=False)

    disp_d = nc.declare_dram_parameter("dispatch", [TPC * E, CAP], fp32, isOutput=True)
    comb_d = nc.declare_dram_parameter("combine", [TPC * E, CAP], fp32, isOutput=True)
    prob_d = nc.declare_dram_parameter("probs", [TPC, E], fp32, isOutput=True)
    imp_d = nc.declare_dram_parameter("importance", [TPC, 1], fp32, isOutput=True)
    aux_d = nc.declare_dram_parameter("aux", [1, 1], fp32, isOutput=True)

    IOA = bass.IndirectOffsetOnAxis
    RG = [list(range(NCORES))]

    with tile.TileContext(nc) as tc, ExitStack() as ctx:
        const = ctx.enter_context(tc.tile_pool(name="const", bufs=1))
        wpool = ctx.enter_context(tc.tile_pool(name="wpool", bufs=1))
        persist = ctx.enter_context(tc.tile_pool(name="persist", bufs=1))
        xraw_p = ctx.enter_context(tc.tile_pool(name="xraw", bufs=2))
        sm = ctx.enter_context(tc.tile_pool(name="sm", bufs=3))       # small transients
        l1ps = ctx.enter_context(tc.tile_pool(name="l1ps", bufs=2, space="PSUM"))
        l2ps = ctx.enter_context(tc.tile_pool(name="l2ps", bufs=2, space="PSUM"))
        ccps_p = ctx.enter_context(tc.tile_pool(name="ccps", bufs=2, space="PSUM"))
        dram = ctx.enter_context(tc.tile_pool(name="dram", bufs=1, space="DRAM"))

        def ts(out, in0, s1, op0, s2=None, op1=None, eng=None):
            e = eng if eng is not None else nc.vector
            if s2 is None:
                e.tensor_scalar(out=out, in0=in0, scalar1=s1, scalar2=None, op0=op0)
            else:
                e.tensor_scalar(out=out, in0=in0, scalar1=s1, scalar2=s2,
                                op0=op0, op1=op1)

        def tt(out, a, b, op):
            nc.vector.tensor_tensor(out=out, in0=a, in1=b, op=op)

        # ---------------- constants (host-provided; no gpsimd on startup) ----
        cbig = const.tile([P, 266], fp32)
        nc.sync.dma_start(out=cbig[:], in_=cc_d[:, :])
        ident = cbig[:, 0:P]
        ut = cbig[:, P:2 * P]
        iota8 = cbig[:, 2 * P:2 * P + E]
        iota_tok = cbig[:, 264:265]
        ones_col = cbig[:, 265:266]
        ones8 = cbig[0:NCORES, 265:266]
        onesrow_t = const.tile([1, TPC], fp32)
        nc.sync.dma_start(out=onesrow_t[:], in_=cr_d[:, :])
        ones_row = onesrow_t[:, 0:P]
        ones_row512 = onesrow_t[:, :]
        zero_sb = const.tile([P, 2048], fp32)
        nc.vector.memset(zero_sb[:], 0.0)

        # warm up the collective path early (result unused)
        wu_in_d = dram.tile([1, 2 * E], fp32)
        wu_out_d = dram.tile([NCORES, 2 * E], fp32, addr_space="Shared")
        wu_sb = const.tile([NCORES, 2 * E], fp32)
        nc.gpsimd.dma_start(out=wu_in_d[:], in_=onesrow_t[:, 0:2 * E])
        nc.gpsimd.collective_compute(
            "AllGather", mybir.AluOpType.bypass,
            ins=[wu_in_d[:]], outs=[wu_out_d[:]], replica_groups=RG,
        )
        nc.gpsimd.dma_start(out=wu_sb[:], in_=wu_out_d[:])

        # ---------------- x load + transpose (PE critical path first) -----
        xT = wpool.tile([P, 8, TPC], fp32)
        for tcch in range(NCH):
            x_raw = xraw_p.tile([P, H], fp32, tag="xraw")
            nc.sync.dma_start(out=x_raw[:], in_=x_d[tcch * P:(tcch + 1) * P, :])
            for kc in range(8):
                tps = l2ps.tile([P, P], fp32, tag="l2")
                nc.tensor.transpose(out=tps[:], in_=x_raw[:, kc * P:(kc + 1) * P],
                                    identity=ident)
                nc.vector.tensor_copy(out=xT[:, kc, tcch * P:(tcch + 1) * P], in_=tps[:])

        # ---------------- weight loads ----------------
        wi1_sb = wpool.tile([P, 8, H2], fp32)
        wr1_sb = wpool.tile([P, 8, H], fp32)
        wu1_sb = wpool.tile([P, 8, H], fp32)
        for kc in range(8):
            nc.sync.dma_start(out=wi1_sb[:, kc, :], in_=wi1_d[kc * P:(kc + 1) * P, :])
            nc.sync.dma_start(out=wr1_sb[:, kc, :], in_=wr1_d[kc * P:(kc + 1) * P, :])
            nc.sync.dma_start(out=wu1_sb[:, kc, :], in_=wu1_d[kc * P:(kc + 1) * P, :])
        wi2_sb = wpool.tile([P, 4, 1], fp32)
        nc.sync.dma_start(out=wi2_sb[:], in_=wi2_d[:, :].rearrange("(k p) m -> p k m", p=P))
        wr2_sb = wpool.tile([P, 8, E], fp32)
        nc.sync.dma_start(out=wr2_sb[:], in_=wr2_d[:, :].rearrange("(k p) m -> p k m", p=P))
        wu2_sb = wpool.tile([P, 8, E], fp32)
        nc.sync.dma_start(out=wu2_sb[:], in_=wu2_d[:, :].rearrange("(k p) m -> p k m", p=P))
        bi1c = wpool.tile([P, 4], fp32)
        nc.sync.dma_start(out=bi1c[:], in_=bi1_d[:].rearrange("(m p) -> p m", p=P))
        br1c = wpool.tile([P, 8], fp32)
        nc.sync.dma_start(out=br1c[:], in_=br1_d[:].rearrange("(m p) -> p m", p=P))
        bu1c = wpool.tile([P, 8], fp32)
        nc.sync.dma_start(out=bu1c[:], in_=bu1_d[:].rearrange("(m p) -> p m", p=P))
        br2row = wpool.tile([1, E], fp32)
        nc.sync.dma_start(out=br2row[:], in_=br2_d[:, :])
        bu2row = wpool.tile([1, E], fp32)
        nc.sync.dma_start(out=bu2row[:], in_=bu2_d[:, :])
        bi2row = wpool.tile([1, 1], fp32)
        nc.sync.dma_start(out=bi2row[:], in_=bi2_d[:, :])
        cmask_sb = wpool.tile([NCORES, 1], fp32)
        nc.sync.dma_start(out=cmask_sb[:], in_=cm_d[:, :])

        # ---------------- zero-fill dispatch/combine (fills DMA idle) -----
        zfill_insts = {"dispatch": [], "combine": []}
        for name, dd in (("dispatch", disp_d), ("combine", comb_d)):
            flat = dd[:, :].rearrange("r c -> (r c)")
            chunk = P * 2048
            for i in range((TPC * E * CAP) // chunk):   # 24 x 1MiB each
                ins = nc.sync.dma_start(
                    out=flat[i * chunk:(i + 1) * chunk].rearrange("(p f) -> p f", p=P),
                    in_=zero_sb[:],
                )
                zfill_insts[name].append(ins)

        # ---------------- persistent small tensors ----------------
        mask_sb = persist.tile([P, NCH], fp32)
        li_sb = persist.tile([P, NCH, E], fp32)
        logits_sb = persist.tile([P, NCH, E], fp32)
        rmax_sb = persist.tile([P, NCH], fp32)
        probs_sb = persist.tile([P, NCH, E], fp32)
        ohs = persist.tile([P, NCH, 2, E], fp32)
        idx_sb = persist.tile([P, NCH, 2], fp32)
        w_sb = persist.tile([P, NCH, 2], fp32)
        prefix_sb = persist.tile([P, NCH, 2, E], fp32)  # within-chunk excl prefix
        chcnt = persist.tile([1, 2, NCH, E], fp32)   # per (k, chunk) counts
        carr = persist.tile([1, 2, NCH, E], fp32)    # per (k, chunk) carry
        probsum_sb = persist.tile([1, E], fp32)
        impsum_sb = persist.tile([1, E], fp32)
        agin2 = persist.tile([1, 2 * E], fp32)       # aux AG payload
        cnt_all = persist.tile([NCORES, 2 * NCH * E], fp32)
        tot_k = persist.tile([NCORES, 2, E], fp32)
        aux_all = persist.tile([NCORES, 2 * E], fp32)
        off0 = persist.tile([1, E], fp32)
        off1 = persist.tile([1, E], fp32)
        tot0 = persist.tile([1, E], fp32)
        addv = persist.tile([1, NCH * 2 * E], fp32)  # carry+offset per (tc,k)
        rppe_sb = persist.tile([1, E], fp32)
        ims_sb = persist.tile([1, E], fp32)

        # ---------------- layer 1 helper ----------------
        def layer1(w1_sb, nmc, bias_col):
            hb = wpool.tile([P, 8, TPC], fp32, tag="hbuf", name="hbuf")
            for mc in range(nmc):
                ps = l1ps.tile([P, TPC], fp32, tag="l1")
                for kc in range(8):
                    nc.tensor.matmul(out=ps[:], lhsT=w1_sb[:, kc, mc * P:(mc + 1) * P],
                                     rhs=xT[:, kc, :], start=(kc == 0), stop=(kc == 7))
                nc.scalar.activation(out=hb[:, mc, :], in_=ps[:], func=AF.Relu,
                                     bias=bias_col[:, mc:mc + 1], scale=1.0)
            return hb

        # --- importance net (batched layer 2: [1, 512] row) ---
        hbuf = layer1(wi1_sb, 4, bi1c)
        zrow_ps = l2ps.tile([1, TPC], fp32, tag="l2row")
        for kc in range(4):
            nc.tensor.matmul(out=zrow_ps[:], lhsT=wi2_sb[:, kc, :],
                             rhs=hbuf[:, kc, :], start=(kc == 0), stop=False)
        nc.tensor.matmul(out=zrow_ps[:], lhsT=bi2row[:, :], rhs=ones_row512,
                         start=False, stop=True)
        zrow = persist.tile([1, TPC], fp32)
        nc.vector.tensor_copy(out=zrow[:], in_=zrow_ps[:])
        for tcch in range(NCH):
            zps = l2ps.tile([P, 1], fp32, tag="l2")
            nc.tensor.transpose(out=zps[:], in_=zrow[:, tcch * P:(tcch + 1) * P],
                                identity=ident[0:1, 0:1])
            ts(mask_sb[:, tcch:tcch + 1], zps[:], 0.0, OP.is_gt)
            impv = sm.tile([P, 1], fp32, tag="impv")
            nc.scalar.activation(out=impv[:], in_=zps[:], func=AF.Sigmoid)
            nc.sync.dma_start(out=imp_d[tcch * P:(tcch + 1) * P, :], in_=impv[:])

        # --- router_important (batched layer 2: [E, 512] rows) ---
        hbuf = layer1(wr1_sb, 8, br1c)
        lirow_ps = l2ps.tile([E, TPC], fp32, tag="l2row")
        for kc in range(8):
            nc.tensor.matmul(out=lirow_ps[:], lhsT=wr2_sb[:, kc, :],
                             rhs=hbuf[:, kc, :], start=(kc == 0), stop=False)
        nc.tensor.matmul(out=lirow_ps[:], lhsT=br2row[:, :], rhs=ones_row512,
                         start=False, stop=True)
        lirow = persist.tile([E, TPC], fp32)
        nc.vector.tensor_copy(out=lirow[:], in_=lirow_ps[:])
        for tcch in range(NCH):
            lps = l2ps.tile([P, E], fp32, tag="l2")
            nc.tensor.transpose(out=lps[:], in_=lirow[:, tcch * P:(tcch + 1) * P],
                                identity=ident[0:E, 0:E])
            nc.vector.tensor_copy(out=li_sb[:, tcch, :], in_=lps[:])

        # --- router_unimportant + decision chain (everything the counts
        #     AllGather needs: logits -> top2 -> one-hots -> counts) ---
        hbuf = layer1(wu1_sb, 8, bu1c)
        lurow_ps = l2ps.tile([E, TPC], fp32, tag="l2row")
        for kc in range(8):
            nc.tensor.matmul(out=lurow_ps[:], lhsT=wu2_sb[:, kc, :],
                             rhs=hbuf[:, kc, :], start=(kc == 0), stop=False)
        nc.tensor.matmul(out=lurow_ps[:], lhsT=bu2row[:, :], rhs=ones_row512,
                         start=False, stop=True)
        lurow = persist.tile([E, TPC], fp32)
        nc.vector.tensor_copy(out=lurow[:], in_=lurow_ps[:])
        for tcch in range(NCH):
            tsl = slice(tcch * P, (tcch + 1) * P)
            lups = l2ps.tile([P, E], fp32, tag="l2")
            nc.tensor.transpose(out=lups[:], in_=lurow[:, tsl],
                                identity=ident[0:E, 0:E])

            m = mask_sb[:, tcch:tcch + 1]
            invm = sm.tile([P, 1], fp32, tag="invm")
            ts(invm[:], m, -1.0, OP.mult, 1.0, OP.add)
            t1 = sm.tile([P, E], fp32, tag="t1")
            ts(t1[:], li_sb[:, tcch, :], m, OP.mult)
            t2 = sm.tile([P, E], fp32, tag="t2")
            ts(t2[:], lups[:], invm[:], OP.mult)
            tt(logits_sb[:, tcch, :], t1[:], t2[:], OP.add)
            logits = logits_sb[:, tcch, :]

            # top-2 on logits (exact fp32)
            rmax = rmax_sb[:, tcch:tcch + 1]
            nc.vector.tensor_reduce(out=rmax, in_=logits, axis=AX.X, op=OP.max)
            eq1 = sm.tile([P, E], fp32, tag="eq1")
            ts(eq1[:], logits, rmax, OP.is_equal)
            b1 = sm.tile([P, E], fp32, tag="b1")
            ts(b1[:], eq1[:], -999.0, OP.mult, 999.0, OP.add)
            c1 = sm.tile([P, E], fp32, tag="c1")
            tt(c1[:], b1[:], iota8, OP.add)
            nc.vector.tensor_reduce(out=idx_sb[:, tcch, 0:1], in_=c1[:], axis=AX.X,
                                    op=OP.min)
            ts(ohs[:, tcch, 0, :], iota8, idx_sb[:, tcch, 0:1], OP.is_equal)
            negm = sm.tile([P, E], fp32, tag="negm")
            ts(negm[:], ohs[:, tcch, 0, :], -1.0e9, OP.mult)
            lm = sm.tile([P, E], fp32, tag="lm")
            tt(lm[:], logits, negm[:], OP.add)
            l2v = sm.tile([P, 1], fp32, tag="l2v")
            nc.vector.tensor_reduce(out=l2v[:], in_=lm[:], axis=AX.X, op=OP.max)
            eq2 = sm.tile([P, E], fp32, tag="eq2")
            ts(eq2[:], lm[:], l2v[:], OP.is_equal)
            b2 = sm.tile([P, E], fp32, tag="b2")
            ts(b2[:], eq2[:], -999.0, OP.mult, 999.0, OP.add)
            c2 = sm.tile([P, E], fp32, tag="c2")
            tt(c2[:], b2[:], iota8, OP.add)
            nc.vector.tensor_reduce(out=idx_sb[:, tcch, 1:2], in_=c2[:], axis=AX.X,
                                    op=OP.min)
            ts(ohs[:, tcch, 1, :], iota8, idx_sb[:, tcch, 1:2], OP.is_equal)

            for k in range(2):
                ccp = ccps_p.tile([1, E], fp32, tag="ccp")
                nc.tensor.matmul(out=ccp[:], lhsT=ones_col,
                                 rhs=ohs[:, tcch, k, :], start=True, stop=True)
                nc.vector.tensor_copy(out=chcnt[:, k, tcch, :], in_=ccp[:])

        # fire the counts AllGather with the raw per-chunk counts ASAP
        ag_in_d = dram.tile([1, 2 * NCH * E], fp32)
        ag_out_d = dram.tile([NCORES, 2 * NCH * E], fp32, addr_space="Shared")
        nc.gpsimd.dma_start(out=ag_in_d[:], in_=chcnt[:, :, :, :])
        nc.gpsimd.collective_compute(
            "AllGather", mybir.AluOpType.bypass,
            ins=[ag_in_d[:]], outs=[ag_out_d[:]], replica_groups=RG,
        )
        nc.gpsimd.dma_start(out=cnt_all[:], in_=ag_out_d[:])

        # my own chunk carries (overlaps the AllGather)
        for k in range(2):
            nc.vector.tensor_copy(out=carr[:, k, 1, :], in_=chcnt[:, k, 0, :])
            tt(carr[:, k, 2, :], carr[:, k, 1, :], chcnt[:, k, 1, :], OP.add)
            tt(carr[:, k, 3, :], carr[:, k, 2, :], chcnt[:, k, 2, :], OP.add)

        # --- value chain per chunk (overlaps the AllGather): softmax,
        #     gate weights, masked-prob sums, prefix matmuls ---
        for tcch in range(NCH):
            tsl = slice(tcch * P, (tcch + 1) * P)
            logits = logits_sb[:, tcch, :]
            nrmax = sm.tile([P, 1], fp32, tag="nrmax")
            ts(nrmax[:], rmax_sb[:, tcch:tcch + 1], -1.0, OP.mult)
            exp_t = sm.tile([P, E], fp32, tag="exp_t")
            ssum = sm.tile([P, 1], fp32, tag="ssum")
            nc.scalar.activation(out=exp_t[:], in_=logits, func=AF.Exp,
                                 bias=nrmax[:], scale=1.0, accum_out=ssum[:])
            rs0 = sm.tile([P, 1], fp32, tag="rs0")
            nc.vector.reciprocal(out=rs0[:], in_=ssum[:])
            nt1 = sm.tile([P, 1], fp32, tag="nt1")
            tt(nt1[:], ssum[:], rs0[:], OP.mult)
            nt2 = sm.tile([P, 1], fp32, tag="nt2")
            ts(nt2[:], nt1[:], -1.0, OP.mult, 2.0, OP.add)
            rs = sm.tile([P, 1], fp32, tag="rs")
            tt(rs[:], rs0[:], nt2[:], OP.mult)
            ts(probs_sb[:, tcch, :], exp_t[:], rs[:], OP.mult)
            nc.sync.dma_start(out=prob_d[tsl, :], in_=probs_sb[:, tcch, :])

            scr1 = sm.tile([P, E], fp32, tag="scr1")
            p1 = sm.tile([P, 1], fp32, tag="p1")
            tt(scr1[:], probs_sb[:, tcch, :], ohs[:, tcch, 0, :], OP.mult)
            nc.vector.tensor_reduce(out=p1[:], in_=scr1[:], axis=AX.X, op=OP.add)
            scr2 = sm.tile([P, E], fp32, tag="scr2")
            p2 = sm.tile([P, 1], fp32, tag="p2")
            tt(scr2[:], probs_sb[:, tcch, :], ohs[:, tcch, 1, :], OP.mult)
            nc.vector.tensor_reduce(out=p2[:], in_=scr2[:], axis=AX.X, op=OP.add)
            ws = sm.tile([P, 1], fp32, tag="ws")
            tt(ws[:], p1[:], p2[:], OP.add)
            rw0 = sm.tile([P, 1], fp32, tag="rw0")
            nc.vector.reciprocal(out=rw0[:], in_=ws[:])
            wt1 = sm.tile([P, 1], fp32, tag="wt1")
            tt(wt1[:], ws[:], rw0[:], OP.mult)
            wt2 = sm.tile([P, 1], fp32, tag="wt2")
            ts(wt2[:], wt1[:], -1.0, OP.mult, 2.0, OP.add)
            rw = sm.tile([P, 1], fp32, tag="rw")
            tt(rw[:], rw0[:], wt2[:], OP.mult)
            tt(w_sb[:, tcch, 0:1], p1[:], rw[:], OP.mult)
            tt(w_sb[:, tcch, 1:2], p2[:], rw[:], OP.mult)

            pm = sm.tile([P, E], fp32, tag="pm")
            ts(pm[:], probs_sb[:, tcch, :], mask_sb[:, tcch:tcch + 1], OP.mult)
            pst = ccps_p.tile([1, E], fp32, tag="ccp")
            nc.tensor.matmul(out=pst[:], lhsT=ones_col,
                             rhs=probs_sb[:, tcch, :], start=True, stop=True)
            if tcch == 0:
                nc.vector.tensor_copy(out=probsum_sb[:], in_=pst[:])
            else:
                tt(probsum_sb[:], probsum_sb[:], pst[:], OP.add)
            ist = ccps_p.tile([1, E], fp32, tag="ccp")
            nc.tensor.matmul(out=ist[:], lhsT=ones_col,
                             rhs=pm[:], start=True, stop=True)
            if tcch == 0:
                nc.vector.tensor_copy(out=impsum_sb[:], in_=ist[:])
            else:
                tt(impsum_sb[:], impsum_sb[:], ist[:], OP.add)

            for k in range(2):
                pfx = l2ps.tile([P, E], fp32, tag="l2")
                nc.tensor.matmul(out=pfx[:], lhsT=ut,
                                 rhs=ohs[:, tcch, k, :], start=True, stop=True)
                nc.vector.tensor_copy(out=prefix_sb[:, tcch, k, :], in_=pfx[:])

        # second AllGather for the aux partial sums (overlaps scatter tail)
        nc.vector.tensor_copy(out=agin2[:, 0:E], in_=probsum_sb[:])
        nc.vector.tensor_copy(out=agin2[:, E:2 * E], in_=impsum_sb[:])
        ag2_in_d = dram.tile([1, 2 * E], fp32)
        ag2_out_d = dram.tile([NCORES, 2 * E], fp32, addr_space="Shared")
        nc.gpsimd.dma_start(out=ag2_in_d[:], in_=agin2[:])
        nc.gpsimd.collective_compute(
            "AllGather", mybir.AluOpType.bypass,
            ins=[ag2_in_d[:]], outs=[ag2_out_d[:]], replica_groups=RG,
        )
        nc.gpsimd.dma_start(out=aux_all[:], in_=ag2_out_d[:])

        # per-core totals per k (reduce the chunk axis), then offsets
        for k in range(2):
            nc.vector.tensor_reduce(
                out=tot_k[:, k, :],
                in_=cnt_all[:, k * NCH * E:(k + 1) * NCH * E]
                    .rearrange("p (n e) -> p e n", n=NCH),
                axis=AX.X, op=OP.add)
        ms0 = ccps_p.tile([1, E], fp32, tag="ccp")
        nc.tensor.matmul(out=ms0[:], lhsT=cmask_sb[:, :], rhs=tot_k[:, 0, :],
                         start=True, stop=True)
        nc.vector.tensor_copy(out=off0[:], in_=ms0[:])
        t0p = ccps_p.tile([1, E], fp32, tag="ccp")
        nc.tensor.matmul(out=t0p[:], lhsT=ones8, rhs=tot_k[:, 0, :],
                         start=True, stop=True)
        nc.vector.tensor_copy(out=tot0[:], in_=t0p[:])
        ms1 = ccps_p.tile([1, E], fp32, tag="ccp")
        nc.tensor.matmul(out=ms1[:], lhsT=cmask_sb[:, :], rhs=tot_k[:, 1, :],
                         start=True, stop=True)
        tt(off1[:], ms1[:], tot0[:], OP.add)

        for tcch in range(NCH):
            for k in range(2):
                sl = (tcch * 2 + k) * E
                offk = off0 if k == 0 else off1
                if tcch == 0:
                    nc.vector.tensor_copy(out=addv[:, sl:sl + E], in_=offk[:])
                else:
                    tt(addv[:, sl:sl + E], carr[:, k, tcch, :], offk[:], OP.add)
        bc_ps = l2ps.tile([P, NCH * 2 * E], fp32, tag="l2")
        nc.tensor.matmul(out=bc_ps[:], lhsT=ones_row, rhs=addv[:, :],
                         start=True, stop=True)

        # final positions + single-element scatter: flat index
        # (t*E + e)*CAP + pos; dispatch writes 1.0, combine writes w.
        disp_flat = disp_d[:, :].rearrange("r c -> (r c)")[:, None]
        comb_flat = comb_d[:, :].rearrange("r c -> (r c)")[:, None]
        for tcch in range(NCH):
            for k in range(2):
                sl = (tcch * 2 + k) * E
                padd = sm.tile([P, E], fp32, tag="padd")
                tt(padd[:], prefix_sb[:, tcch, k, :], bc_ps[:, sl:sl + E], OP.add)
                scr = sm.tile([P, E], fp32, tag="scr")
                posk = sm.tile([P, 1], fp32, tag="posk")
                tt(scr[:], padd[:], ohs[:, tcch, k, :], OP.mult)
                nc.vector.tensor_reduce(out=posk[:], in_=scr[:], axis=AX.X, op=OP.add)
                keep = sm.tile([P, 1], fp32, tag="keep")
                ts(keep[:], posk[:], float(CAP), OP.is_lt)
                posc = sm.tile([P, 1], fp32, tag="posc")
                ts(posc[:], posk[:], float(CAP - 1), OP.min)
                r0 = sm.tile([P, 1], fp32, tag="r0")
                ts(r0[:], idx_sb[:, tcch, k:k + 1], float(CAP), OP.mult,
                   float(tcch * P * E * CAP), OP.add)
                r1 = sm.tile([P, 1], fp32, tag="r1")
                tt(r1[:], r0[:], iota_tok, OP.add)
                r2 = sm.tile([P, 1], fp32, tag="r2")
                tt(r2[:], r1[:], posc[:], OP.add)
                nk = sm.tile([P, 1], fp32, tag="nk")
                ts(nk[:], keep[:], -BIG, OP.mult, BIG, OP.add)
                rf = sm.tile([P, 1], fp32, tag="rf")
                tt(rf[:], r2[:], nk[:], OP.add)
                ri = sm.tile([P, 1], i32, tag="ri")
                nc.vector.tensor_copy(out=ri[:], in_=rf[:])

                s1 = nc.gpsimd.indirect_dma_start(
                    out=disp_flat, out_offset=IOA(ap=ri[:, 0:1], axis=0),
                    in_=ones_col, in_offset=None,
                    bounds_check=TPC * E * CAP - 1, oob_is_err=False)
                s2 = nc.gpsimd.indirect_dma_start(
                    out=comb_flat, out_offset=IOA(ap=ri[:, 0:1], axis=0),
                    in_=w_sb[:, tcch, k:k + 1], in_offset=None,
                    bounds_check=TPC * E * CAP - 1, oob_is_err=False)
                for z in zfill_insts["dispatch"]:
                    add_dep_helper(s1.ins, z.ins, reason="scatter after zero-fill")
                for z in zfill_insts["combine"]:
                    add_dep_helper(s2.ins, z.ins, reason="scatter after zero-fill")

        # ---------------- aux loss (from the aux AllGather) ----------------
        def ln_series(out_sb, d_sb, tagp):
            # ln(1+d) = d*(1 - d*(1/2 - d*(1/3 - ...)))
            s = persist.tile([1, E], fp32, name=f"{tagp}_s")
            nc.vector.memset(s[:], 1.0 / NTERMS)
            for i in range(NTERMS - 1, 0, -1):
                mtmp = sm.tile([1, E], fp32, tag=f"{tagp}_m")
                tt(mtmp[:], d_sb[:], s[:], OP.mult)
                ts(s[:], mtmp[:], -1.0, OP.mult, 1.0 / i, OP.add)
            tt(out_sb, d_sb[:], s[:], OP.mult)

        rpp = ccps_p.tile([1, E], fp32, tag="ccp")
        nc.tensor.matmul(out=rpp[:], lhsT=ones8, rhs=aux_all[:, 0:E],
                         start=True, stop=True)
        ts(rppe_sb[:], rpp[:], 1.0 / N, OP.mult)
        imt = ccps_p.tile([1, E], fp32, tag="ccp")
        nc.tensor.matmul(out=imt[:], lhsT=ones8, rhs=aux_all[:, E:2 * E],
                         start=True, stop=True)
        ts(ims_sb[:], imt[:], 1.0e-9, OP.add)

        t8 = persist.tile([1, E], fp32)
        ts(t8[:], rppe_sb[:], 8.0, OP.mult, 1.0e-9, OP.add)
        d1 = persist.tile([1, E], fp32)
        ts(d1[:], t8[:], -1.0, OP.add)
        ln1 = persist.tile([1, E], fp32)
        ln_series(ln1[:], d1, "lnA")
        elt = persist.tile([1, E], fp32)
        tt(elt[:], rppe_sb[:], ln1[:], OP.mult)
        el = persist.tile([1, 1], fp32)
        nc.vector.tensor_reduce(out=el[:], in_=elt[:], axis=AX.X, op=OP.add)

        st = persist.tile([1, 1], fp32)
        nc.vector.tensor_reduce(out=st[:], in_=ims_sb[:], axis=AX.X, op=OP.add)
        rst0 = persist.tile([1, 1], fp32)
        nc.vector.reciprocal(out=rst0[:], in_=st[:])
        at1 = persist.tile([1, 1], fp32)
        tt(at1[:], st[:], rst0[:], OP.mult)
        at2 = persist.tile([1, 1], fp32)
        ts(at2[:], at1[:], -1.0, OP.mult, 2.0, OP.add)
        rst = persist.tile([1, 1], fp32)
        tt(rst[:], rst0[:], at2[:], OP.mult)
        ippe = persist.tile([1, E], fp32)
        ts(ippe[:], ims_sb[:], rst[:], OP.mult)
        u8 = persist.tile([1, E], fp32)
        ts(u8[:], ippe[:], 8.0, OP.mult, 8.0e-9, OP.add)
        d2 = persist.tile([1, E], fp32)
        ts(d2[:], u8[:], -1.0, OP.add)
        ln2 = persist.tile([1, E], fp32)
        ln_series(ln2[:], d2, "lnB")
        lnip = persist.tile([1, E], fp32)
        ts(lnip[:], ln2[:], -LN8, OP.add)
        iet = persist.tile([1, E], fp32)
        tt(iet[:], ippe[:], lnip[:], OP.mult)
        ies = persist.tile([1, 1], fp32)
        nc.vector.tensor_reduce(out=ies[:], in_=iet[:], axis=AX.X, op=OP.add)
        # aux = el + (0.1/ln8) * sum(ippe*ln(ippe+eps))   [ies = -imp_entropy]
        sc = persist.tile([1, 1], fp32)
        ts(sc[:], ies[:], 0.1 / LN8, OP.mult)
        auxv = persist.tile([1, 1], fp32)
        tt(auxv[:], el[:], sc[:], OP.add)
        nc.sync.dma_start(out=aux_d[:, :], in_=auxv[:])

    nc.compile()
    return nc




def _host_consts():
    if "cconst" not in _CACHE:
        cb = np.zeros((P, 266), np.float32)
        cb[:, 0:P] = np.eye(P, dtype=np.float32)
        cb[:, P:2 * P] = np.triu(np.ones((P, P), np.float32), 1)
        cb[:, 2 * P:2 * P + E] = np.tile(np.arange(E, dtype=np.float32), (P, 1))
        cb[:, 264] = np.arange(P, dtype=np.float32) * (E * CAP)
        cb[:, 265] = 1.0
        _CACHE["cconst"] = cb
    return _CACHE["cconst"]

def _get_nc():
    if "nc" not in _CACHE:
        _CACHE["nc"] = _build()
    return _CACHE["nc"]


def kernel(**inputs):
    global LAST_EXEC_NS, LAST_TRACE_DIR
    from concourse.bass_utils import run_bass_kernel_spmd

    inp = {k: np.ascontiguousarray(np.asarray(v), dtype=np.float32)
           for k, v in inputs.items()}
    x = inp["hidden_states"].reshape(N, H)

    nc = _get_nc()
    in_maps = []
    for c in range(NCORES):
        in_maps.append(dict(
            hidden_states=np.ascontiguousarray(x[c * TPC:(c + 1) * TPC]),
            wi1=inp["wi1"], bi1=inp["bi1"], wi2=inp["wi2"],
            bi2=inp["bi2"].reshape(1, 1),
            wr1=inp["wr1"], br1=inp["br1"], wr2=inp["wr2"],
            br2=inp["br2"].reshape(1, E),
            wu1=inp["wu1"], bu1=inp["bu1"], wu2=inp["wu2"],
            bu2=inp["bu2"].reshape(1, E),
            cmask=(np.arange(NCORES) < c).astype(np.float32).reshape(NCORES, 1),
            cconst=_host_consts(),
            crow=np.ones((1, TPC), np.float32),
        ))

    trace = bool(int(os.environ.get("KERNEL_TRACE", "0")))
    kwargs = {}
    if trace:
        _install_ntff_hook()
        import tempfile
        LAST_TRACE_DIR = tempfile.mkdtemp(prefix="adaptive_router_trace_")
        kwargs["tmpdir"] = LAST_TRACE_DIR
    res = run_bass_kernel_spmd(nc, in_maps, core_ids=list(range(NCORES)),
                               trace=trace, **kwargs)
    LAST_EXEC_NS = res.exec_time_ns

    disp = np.concatenate(
        [res.results[c]["dispatch"].reshape(TPC, E, CAP) for c in range(NCORES)], 0
    ).reshape(B, S, E, CAP)
    comb = np.concatenate(
        [res.results[c]["combine"].reshape(TPC, E, CAP) for c in range(NCORES)], 0
    ).reshape(B, S, E, CAP)
    probs = np.concatenate(
        [res.results[c]["probs"] for c in range(NCORES)], 0).reshape(B, S, E)
    impv = np.concatenate(
        [res.results[c]["importance"] for c in range(NCORES)], 0).reshape(B, S, 1)
    aux = np.float32(res.results[0]["aux"].reshape(()))
    return disp, comb, probs, aux, impv


# revision 22
# speedup vs baseline: 2.2939x; 1.0370x over previous
"""Trainium2 Bass kernel for nn_AdaptiveRouter (MoE dual-gate routing).

8 NeuronCores, data-parallel over tokens. Each core handles 512 tokens:
  - fp32 matmuls for importance MLP + both routers (decisions are
    numerically sensitive: mask threshold and top-k tie distances)
  - top-2 selection via exact fp32 compares on logits
  - capacity positions: strict-upper-triangular prefix matmuls within a
    128-token chunk, chunk carries, and an 8-core AllGather of per-core
    per-(k,expert) counts for the global k-major cumsum offsets
  - dispatch/combine [512,8,1536] f32 shards: zero-filled with large
    DMA stores, then one 1536-float row per (token,k) scattered via
    indirect DMA (OOB row index drops capacity-overflow entries)
  - second AllGather for aux-loss partial sums; ln() via series
"""

import math
import os
import sys
import types

sys.path.insert(0, "/opt/trn_rl_repo")

import numpy as np

# ---- problem constants (hardcoded; kernel.py must be self-contained) ----
B, S, H, E, TOPK, CF = 2, 2048, 1024, 8, 2, 1.5
N = B * S                      # 4096 tokens
NCORES = 8
TPC = N // NCORES              # 512 tokens per core
CAP = int(N * CF * TOPK / E)   # 1536
P = 128
NCH = TPC // P                 # 4 token chunks per core
H2 = H // 2
BIG = 8.0e6                    # element marker for dropped entries (OOB -> skipped)
LN8 = math.log(8.0)
NTERMS = 12                    # ln(1+x) series terms

LAST_EXEC_NS = None
LAST_TRACE_DIR = None

_CACHE = {}


def _install_ntff_hook():
    """Recreate antenv.axon_hooks (absent in this image) so
    run_bass_kernel_spmd(trace=True) can profile via libaxon_pjrt."""
    import antenv

    if "antenv.axon_hooks" not in sys.modules:
        mod = types.ModuleType("antenv.axon_hooks")
        mod._hook = None

        def set_axon_ntff_profile_hook(h):
            mod._hook = h

        def get_axon_ntff_profile_hook():
            return mod._hook

        mod.set_axon_ntff_profile_hook = set_axon_ntff_profile_hook
        mod.get_axon_ntff_profile_hook = get_axon_ntff_profile_hook
        sys.modules["antenv.axon_hooks"] = mod
        antenv.axon_hooks = mod
    mod = sys.modules["antenv.axon_hooks"]
    if mod._hook is None:
        from trn_agent_boot.trn_boot import _ntff_profile_via_ctypes

        mod.set_axon_ntff_profile_hook(
            _ntff_profile_via_ctypes("/opt/axon/libaxon_pjrt.so")
        )


def _build():
    import concourse.bacc as bacc
    import concourse.mybir as mybir
    import concourse.tile as tile
    import concourse.bass as bass
    from concourse.tile import add_dep_helper
    from contextlib import ExitStack

    fp32 = mybir.dt.float32
    i32 = mybir.dt.int32
    AF = mybir.ActivationFunctionType
    OP = mybir.AluOpType
    AX = mybir.AxisListType

    nc = bacc.Bacc(None, target_bir_lowering=False)

    x_d = nc.declare_dram_parameter("hidden_states", [TPC, H], fp32, isOutput=False)
    wi1_d = nc.declare_dram_parameter("wi1", [H, H2], fp32, isOutput=False)
    bi1_d = nc.declare_dram_parameter("bi1", [H2], fp32, isOutput=False)
    wi2_d = nc.declare_dram_parameter("wi2", [H2, 1], fp32, isOutput=False)
    bi2_d = nc.declare_dram_parameter("bi2", [1, 1], fp32, isOutput=False)
    wr1_d = nc.declare_dram_parameter("wr1", [H, H], fp32, isOutput=False)
    br1_d = nc.declare_dram_parameter("br1", [H], fp32, isOutput=False)
    wr2_d = nc.declare_dram_parameter("wr2", [H, E], fp32, isOutput=False)
    br2_d = nc.declare_dram_parameter("br2", [1, E], fp32, isOutput=False)
    wu1_d = nc.declare_dram_parameter("wu1", [H, H], fp32, isOutput=False)
    bu1_d = nc.declare_dram_parameter("bu1", [H], fp32, isOutput=False)
    wu2_d = nc.declare_dram_parameter("wu2", [H, E], fp32, isOutput=False)
    bu2_d = nc.declare_dram_parameter("bu2", [1, E], fp32, isOutput=False)
    cm_d = nc.declare_dram_parameter("cmask", [NCORES, 1], fp32, isOutput=False)
    cc_d = nc.declare_dram_parameter("cconst", [P, 274], fp32, isOutput=False)
    cr_d = nc.declare_dram_parameter("crow", [1, TPC], fp32, isOutput=False)

    disp_d = nc.declare_dram_parameter("dispatch", [TPC * E, CAP], fp32, isOutput=True)
    comb_d = nc.declare_dram_parameter("combine", [TPC * E, CAP], fp32, isOutput=True)
    prob_d = nc.declare_dram_parameter("probs", [TPC, E], fp32, isOutput=True)
    imp_d = nc.declare_dram_parameter("importance", [TPC, 1], fp32, isOutput=True)
    aux_d = nc.declare_dram_parameter("aux", [1, 1], fp32, isOutput=True)

    IOA = bass.IndirectOffsetOnAxis
    RG = [list(range(NCORES))]

    with tile.TileContext(nc) as tc, ExitStack() as ctx:
        const = ctx.enter_context(tc.tile_pool(name="const", bufs=1))
        wpool = ctx.enter_context(tc.tile_pool(name="wpool", bufs=1))
        persist = ctx.enter_context(tc.tile_pool(name="persist", bufs=1))
        xraw_p = ctx.enter_context(tc.tile_pool(name="xraw", bufs=2))
        sm = ctx.enter_context(tc.tile_pool(name="sm", bufs=3))       # small transients
        l1ps = ctx.enter_context(tc.tile_pool(name="l1ps", bufs=2, space="PSUM"))
        l2ps = ctx.enter_context(tc.tile_pool(name="l2ps", bufs=2, space="PSUM"))
        ccps_p = ctx.enter_context(tc.tile_pool(name="ccps", bufs=2, space="PSUM"))
        dram = ctx.enter_context(tc.tile_pool(name="dram", bufs=1, space="DRAM"))

        def ts(out, in0, s1, op0, s2=None, op1=None, eng=None):
            e = eng if eng is not None else nc.vector
            if s2 is None:
                e.tensor_scalar(out=out, in0=in0, scalar1=s1, scalar2=None, op0=op0)
            else:
                e.tensor_scalar(out=out, in0=in0, scalar1=s1, scalar2=s2,
                                op0=op0, op1=op1)

        def tt(out, a, b, op):
            nc.vector.tensor_tensor(out=out, in0=a, in1=b, op=op)

        # ---------------- constants (host-provided; no gpsimd on startup) ----
        cbig = const.tile([P, 274], fp32)
        nc.sync.dma_start(out=cbig[:], in_=cc_d[:, :])
        ident = cbig[:, 0:P]
        ut = cbig[:, P:2 * P]
        iota8 = cbig[:, 2 * P:2 * P + E]
        iota_tok = cbig[:, 264:265]
        ones_col = cbig[:, 265:266]
        tcoff = cbig[:, 266:274]
        ones8 = cbig[0:NCORES, 265:266]
        onesrow_t = const.tile([1, TPC], fp32)
        nc.sync.dma_start(out=onesrow_t[:], in_=cr_d[:, :])
        ones_row = onesrow_t[:, 0:P]
        ones_row512 = onesrow_t[:, :]
        zero_sb = const.tile([P, 2048], fp32)
        nc.vector.memset(zero_sb[:], 0.0)

        # warm up the collective path early (result unused)
        wu_in_d = dram.tile([1, 2 * E], fp32)
        wu_out_d = dram.tile([NCORES, 2 * E], fp32, addr_space="Shared")
        wu_sb = const.tile([NCORES, 2 * E], fp32)
        nc.gpsimd.dma_start(out=wu_in_d[:], in_=onesrow_t[:, 0:2 * E])
        nc.gpsimd.collective_compute(
            "AllGather", mybir.AluOpType.bypass,
            ins=[wu_in_d[:]], outs=[wu_out_d[:]], replica_groups=RG,
        )
        nc.gpsimd.dma_start(out=wu_sb[:], in_=wu_out_d[:])

        # ---------------- x load + transpose (PE critical path first) -----
        xT = wpool.tile([P, 8, TPC], fp32)
        for tcch in range(NCH):
            x_raw = xraw_p.tile([P, H], fp32, tag="xraw")
            nc.sync.dma_start(out=x_raw[:], in_=x_d[tcch * P:(tcch + 1) * P, :])
            for kc in range(8):
                tps = l2ps.tile([P, P], fp32, tag="l2")
                nc.tensor.transpose(out=tps[:], in_=x_raw[:, kc * P:(kc + 1) * P],
                                    identity=ident)
                nc.vector.tensor_copy(out=xT[:, kc, tcch * P:(tcch + 1) * P], in_=tps[:])

        # ---------------- weight loads ----------------
        wi1_sb = wpool.tile([P, 8, H2], fp32)
        wr1_sb = wpool.tile([P, 8, H], fp32)
        wu1_sb = wpool.tile([P, 8, H], fp32)
        for kc in range(8):
            nc.sync.dma_start(out=wi1_sb[:, kc, :], in_=wi1_d[kc * P:(kc + 1) * P, :])
            nc.sync.dma_start(out=wr1_sb[:, kc, :], in_=wr1_d[kc * P:(kc + 1) * P, :])
            nc.sync.dma_start(out=wu1_sb[:, kc, :], in_=wu1_d[kc * P:(kc + 1) * P, :])
        wi2_sb = wpool.tile([P, 4, 1], fp32)
        nc.sync.dma_start(out=wi2_sb[:], in_=wi2_d[:, :].rearrange("(k p) m -> p k m", p=P))
        wr2_sb = wpool.tile([P, 8, E], fp32)
        nc.sync.dma_start(out=wr2_sb[:], in_=wr2_d[:, :].rearrange("(k p) m -> p k m", p=P))
        wu2_sb = wpool.tile([P, 8, E], fp32)
        nc.sync.dma_start(out=wu2_sb[:], in_=wu2_d[:, :].rearrange("(k p) m -> p k m", p=P))
        bi1c = wpool.tile([P, 4], fp32)
        nc.sync.dma_start(out=bi1c[:], in_=bi1_d[:].rearrange("(m p) -> p m", p=P))
        br1c = wpool.tile([P, 8], fp32)
        nc.sync.dma_start(out=br1c[:], in_=br1_d[:].rearrange("(m p) -> p m", p=P))
        bu1c = wpool.tile([P, 8], fp32)
        nc.sync.dma_start(out=bu1c[:], in_=bu1_d[:].rearrange("(m p) -> p m", p=P))
        br2row = wpool.tile([1, E], fp32)
        nc.sync.dma_start(out=br2row[:], in_=br2_d[:, :])
        bu2row = wpool.tile([1, E], fp32)
        nc.sync.dma_start(out=bu2row[:], in_=bu2_d[:, :])
        bi2row = wpool.tile([1, 1], fp32)
        nc.sync.dma_start(out=bi2row[:], in_=bi2_d[:, :])
        cmask_sb = wpool.tile([NCORES, 1], fp32)
        nc.sync.dma_start(out=cmask_sb[:], in_=cm_d[:, :])

        # ---------------- zero-fill dispatch/combine (fills DMA idle) -----
        zfill_insts = {"dispatch": [], "combine": []}
        for name, dd in (("dispatch", disp_d), ("combine", comb_d)):
            flat = dd[:, :].rearrange("r c -> (r c)")
            chunk = P * 2048
            for i in range((TPC * E * CAP) // chunk):   # 24 x 1MiB each
                ins = nc.sync.dma_start(
                    out=flat[i * chunk:(i + 1) * chunk].rearrange("(p f) -> p f", p=P),
                    in_=zero_sb[:],
                )
                zfill_insts[name].append(ins)

        # ---------------- persistent small tensors ----------------
        mask_sb = persist.tile([P, NCH], fp32)
        li_sb = persist.tile([P, NCH, E], fp32)
        logits_sb = persist.tile([P, NCH, E], fp32)
        rmax_sb = persist.tile([P, NCH], fp32)
        probs_sb = persist.tile([P, NCH, E], fp32)
        ohs = persist.tile([P, NCH, 2, E], fp32)
        idx_sb = persist.tile([P, NCH, 2], fp32)
        w_sb = persist.tile([P, NCH, 2], fp32)
        prefix_sb = persist.tile([P, NCH, 2, E], fp32)  # within-chunk excl prefix
        chcnt = persist.tile([1, 2, NCH, E], fp32)   # per (k, chunk) counts
        carr = persist.tile([1, 2, NCH, E], fp32)    # per (k, chunk) carry
        probsum_sb = persist.tile([1, E], fp32)
        impsum_sb = persist.tile([1, E], fp32)
        agin2 = persist.tile([1, 2 * E], fp32)       # aux AG payload
        cnt_all = persist.tile([NCORES, 2 * NCH * E], fp32)
        tot_k = persist.tile([NCORES, 2, E], fp32)
        aux_all = persist.tile([NCORES, 2 * E], fp32)
        off0 = persist.tile([1, E], fp32)
        off1 = persist.tile([1, E], fp32)
        tot0 = persist.tile([1, E], fp32)
        addv = persist.tile([1, NCH * 2 * E], fp32)  # carry+offset per (tc,k)
        ri_all = persist.tile([P, NCH * 2], i32)
        ones_pk = persist.tile([P, NCH * 2], fp32)
        rppe_sb = persist.tile([1, E], fp32)
        ims_sb = persist.tile([1, E], fp32)

        # ---------------- layer 1 helper ----------------
        def layer1(w1_sb, nmc, bias_col, tag="hbuf"):
            hb = wpool.tile([P, 8, TPC], fp32, tag=tag, name="hbuf")
            for mc in range(nmc):
                ps = l1ps.tile([P, TPC], fp32, tag="l1")
                for kc in range(8):
                    nc.tensor.matmul(out=ps[:], lhsT=w1_sb[:, kc, mc * P:(mc + 1) * P],
                                     rhs=xT[:, kc, :], start=(kc == 0), stop=(kc == 7))
                nc.scalar.activation(out=hb[:, mc, :], in_=ps[:], func=AF.Relu,
                                     bias=bias_col[:, mc:mc + 1], scale=1.0)
            return hb

        # --- importance net (batched layer 2: [1, 512] row) ---
        hbuf = layer1(wi1_sb, 4, bi1c, tag='h1buf')
        zrow_ps = l2ps.tile([1, TPC], fp32, tag="l2row")
        for kc in range(4):
            nc.tensor.matmul(out=zrow_ps[:], lhsT=wi2_sb[:, kc, :],
                             rhs=hbuf[:, kc, :], start=(kc == 0), stop=False)
        nc.tensor.matmul(out=zrow_ps[:], lhsT=bi2row[:, :], rhs=ones_row512,
                         start=False, stop=True)
        zrow = persist.tile([1, TPC], fp32)
        nc.vector.tensor_copy(out=zrow[:], in_=zrow_ps[:])
        for tcch in range(NCH):
            zps = l2ps.tile([P, 1], fp32, tag="l2")
            nc.tensor.transpose(out=zps[:], in_=zrow[:, tcch * P:(tcch + 1) * P],
                                identity=ident[0:1, 0:1])
            ts(mask_sb[:, tcch:tcch + 1], zps[:], 0.0, OP.is_gt)
            impv = sm.tile([P, 1], fp32, tag="impv")
            nc.scalar.activation(out=impv[:], in_=zps[:], func=AF.Sigmoid)
            nc.sync.dma_start(out=imp_d[tcch * P:(tcch + 1) * P, :], in_=impv[:])

        # --- router_important (batched layer 2: [E, 512] rows) ---
        hbuf = layer1(wr1_sb, 8, br1c)
        lirow_ps = l2ps.tile([E, TPC], fp32, tag="l2row")
        for kc in range(8):
            nc.tensor.matmul(out=lirow_ps[:], lhsT=wr2_sb[:, kc, :],
                             rhs=hbuf[:, kc, :], start=(kc == 0), stop=False)
        nc.tensor.matmul(out=lirow_ps[:], lhsT=br2row[:, :], rhs=ones_row512,
                         start=False, stop=True)
        lirow = persist.tile([E, TPC], fp32)
        nc.vector.tensor_copy(out=lirow[:], in_=lirow_ps[:])
        for tcch in range(NCH):
            lps = l2ps.tile([P, E], fp32, tag="l2")
            nc.tensor.transpose(out=lps[:], in_=lirow[:, tcch * P:(tcch + 1) * P],
                                identity=ident[0:E, 0:E])
            nc.vector.tensor_copy(out=li_sb[:, tcch, :], in_=lps[:])

        # --- router_unimportant + decision chain (everything the counts
        #     AllGather needs: logits -> top2 -> one-hots -> counts) ---
        hbuf = layer1(wu1_sb, 8, bu1c)
        lurow_ps = l2ps.tile([E, TPC], fp32, tag="l2row")
        for kc in range(8):
            nc.tensor.matmul(out=lurow_ps[:], lhsT=wu2_sb[:, kc, :],
                             rhs=hbuf[:, kc, :], start=(kc == 0), stop=False)
        nc.tensor.matmul(out=lurow_ps[:], lhsT=bu2row[:, :], rhs=ones_row512,
                         start=False, stop=True)
        lurow = persist.tile([E, TPC], fp32)
        nc.vector.tensor_copy(out=lurow[:], in_=lurow_ps[:])
        for tcch in range(NCH):
            tsl = slice(tcch * P, (tcch + 1) * P)
            lups = l2ps.tile([P, E], fp32, tag="l2")
            nc.tensor.transpose(out=lups[:], in_=lurow[:, tsl],
                                identity=ident[0:E, 0:E])

            m = mask_sb[:, tcch:tcch + 1]
            invm = sm.tile([P, 1], fp32, tag="invm")
            ts(invm[:], m, -1.0, OP.mult, 1.0, OP.add)
            t1 = sm.tile([P, E], fp32, tag="t1")
            ts(t1[:], li_sb[:, tcch, :], m, OP.mult)
            t2 = sm.tile([P, E], fp32, tag="t2")
            ts(t2[:], lups[:], invm[:], OP.mult)
            tt(logits_sb[:, tcch, :], t1[:], t2[:], OP.add)
            logits = logits_sb[:, tcch, :]

            # top-2 on logits (exact fp32)
            rmax = rmax_sb[:, tcch:tcch + 1]
            nc.vector.tensor_reduce(out=rmax, in_=logits, axis=AX.X, op=OP.max)
            eq1 = sm.tile([P, E], fp32, tag="eq1")
            ts(eq1[:], logits, rmax, OP.is_equal)
            b1 = sm.tile([P, E], fp32, tag="b1")
            ts(b1[:], eq1[:], -999.0, OP.mult, 999.0, OP.add)
            c1 = sm.tile([P, E], fp32, tag="c1")
            tt(c1[:], b1[:], iota8, OP.add)
            nc.vector.tensor_reduce(out=idx_sb[:, tcch, 0:1], in_=c1[:], axis=AX.X,
                                    op=OP.min)
            ts(ohs[:, tcch, 0, :], iota8, idx_sb[:, tcch, 0:1], OP.is_equal)
            negm = sm.tile([P, E], fp32, tag="negm")
            ts(negm[:], ohs[:, tcch, 0, :], -1.0e9, OP.mult)
            lm = sm.tile([P, E], fp32, tag="lm")
            tt(lm[:], logits, negm[:], OP.add)
            l2v = sm.tile([P, 1], fp32, tag="l2v")
            nc.vector.tensor_reduce(out=l2v[:], in_=lm[:], axis=AX.X, op=OP.max)
            eq2 = sm.tile([P, E], fp32, tag="eq2")
            ts(eq2[:], lm[:], l2v[:], OP.is_equal)
            b2 = sm.tile([P, E], fp32, tag="b2")
            ts(b2[:], eq2[:], -999.0, OP.mult, 999.0, OP.add)
            c2 = sm.tile([P, E], fp32, tag="c2")
            tt(c2[:], b2[:], iota8, OP.add)
            nc.vector.tensor_reduce(out=idx_sb[:, tcch, 1:2], in_=c2[:], axis=AX.X,
                                    op=OP.min)
            ts(ohs[:, tcch, 1, :], iota8, idx_sb[:, tcch, 1:2], OP.is_equal)

            for k in range(2):
                ccp = ccps_p.tile([1, E], fp32, tag="ccp")
                nc.tensor.matmul(out=ccp[:], lhsT=ones_col,
                                 rhs=ohs[:, tcch, k, :], start=True, stop=True)
                nc.vector.tensor_copy(out=chcnt[:, k, tcch, :], in_=ccp[:])

        # fire the counts AllGather with the raw per-chunk counts ASAP
        ag_in_d = dram.tile([1, 2 * NCH * E], fp32)
        ag_out_d = dram.tile([NCORES, 2 * NCH * E], fp32, addr_space="Shared")
        nc.gpsimd.dma_start(out=ag_in_d[:], in_=chcnt[:, :, :, :])
        nc.gpsimd.collective_compute(
            "AllGather", mybir.AluOpType.bypass,
            ins=[ag_in_d[:]], outs=[ag_out_d[:]], replica_groups=RG,
        )
        nc.gpsimd.dma_start(out=cnt_all[:], in_=ag_out_d[:])

        # my own chunk carries (overlaps the AllGather)
        for k in range(2):
            nc.vector.tensor_copy(out=carr[:, k, 1, :], in_=chcnt[:, k, 0, :])
            tt(carr[:, k, 2, :], carr[:, k, 1, :], chcnt[:, k, 1, :], OP.add)
            tt(carr[:, k, 3, :], carr[:, k, 2, :], chcnt[:, k, 2, :], OP.add)

        # --- value chain per chunk (overlaps the AllGather): softmax,
        #     gate weights, masked-prob sums, prefix matmuls ---
        for tcch in range(NCH):
            tsl = slice(tcch * P, (tcch + 1) * P)
            logits = logits_sb[:, tcch, :]
            nrmax = sm.tile([P, 1], fp32, tag="nrmax")
            ts(nrmax[:], rmax_sb[:, tcch:tcch + 1], -1.0, OP.mult)
            exp_t = sm.tile([P, E], fp32, tag="exp_t")
            ssum = sm.tile([P, 1], fp32, tag="ssum")
            nc.scalar.activation(out=exp_t[:], in_=logits, func=AF.Exp,
                                 bias=nrmax[:], scale=1.0, accum_out=ssum[:])
            rs0 = sm.tile([P, 1], fp32, tag="rs0")
            nc.vector.reciprocal(out=rs0[:], in_=ssum[:])
            nt1 = sm.tile([P, 1], fp32, tag="nt1")
            tt(nt1[:], ssum[:], rs0[:], OP.mult)
            nt2 = sm.tile([P, 1], fp32, tag="nt2")
            ts(nt2[:], nt1[:], -1.0, OP.mult, 2.0, OP.add)
            rs = sm.tile([P, 1], fp32, tag="rs")
            tt(rs[:], rs0[:], nt2[:], OP.mult)
            ts(probs_sb[:, tcch, :], exp_t[:], rs[:], OP.mult)
            nc.sync.dma_start(out=prob_d[tsl, :], in_=probs_sb[:, tcch, :])

            scr1 = sm.tile([P, E], fp32, tag="scr1")
            p1 = sm.tile([P, 1], fp32, tag="p1")
            tt(scr1[:], probs_sb[:, tcch, :], ohs[:, tcch, 0, :], OP.mult)
            nc.vector.tensor_reduce(out=p1[:], in_=scr1[:], axis=AX.X, op=OP.add)
            scr2 = sm.tile([P, E], fp32, tag="scr2")
            p2 = sm.tile([P, 1], fp32, tag="p2")
            tt(scr2[:], probs_sb[:, tcch, :], ohs[:, tcch, 1, :], OP.mult)
            nc.vector.tensor_reduce(out=p2[:], in_=scr2[:], axis=AX.X, op=OP.add)
            ws = sm.tile([P, 1], fp32, tag="ws")
            tt(ws[:], p1[:], p2[:], OP.add)
            rw0 = sm.tile([P, 1], fp32, tag="rw0")
            nc.vector.reciprocal(out=rw0[:], in_=ws[:])
            wt1 = sm.tile([P, 1], fp32, tag="wt1")
            tt(wt1[:], ws[:], rw0[:], OP.mult)
            wt2 = sm.tile([P, 1], fp32, tag="wt2")
            ts(wt2[:], wt1[:], -1.0, OP.mult, 2.0, OP.add)
            rw = sm.tile([P, 1], fp32, tag="rw")
            tt(rw[:], rw0[:], wt2[:], OP.mult)
            tt(w_sb[:, tcch, 0:1], p1[:], rw[:], OP.mult)
            tt(w_sb[:, tcch, 1:2], p2[:], rw[:], OP.mult)

            pm = sm.tile([P, E], fp32, tag="pm")
            ts(pm[:], probs_sb[:, tcch, :], mask_sb[:, tcch:tcch + 1], OP.mult)
            pst = ccps_p.tile([1, E], fp32, tag="ccp")
            nc.tensor.matmul(out=pst[:], lhsT=ones_col,
                             rhs=probs_sb[:, tcch, :], start=True, stop=True)
            if tcch == 0:
                nc.vector.tensor_copy(out=probsum_sb[:], in_=pst[:])
            else:
                tt(probsum_sb[:], probsum_sb[:], pst[:], OP.add)
            ist = ccps_p.tile([1, E], fp32, tag="ccp")
            nc.tensor.matmul(out=ist[:], lhsT=ones_col,
                             rhs=pm[:], start=True, stop=True)
            if tcch == 0:
                nc.vector.tensor_copy(out=impsum_sb[:], in_=ist[:])
            else:
                tt(impsum_sb[:], impsum_sb[:], ist[:], OP.add)

            for k in range(2):
                pfx = l2ps.tile([P, E], fp32, tag="l2")
                nc.tensor.matmul(out=pfx[:], lhsT=ut,
                                 rhs=ohs[:, tcch, k, :], start=True, stop=True)
                nc.vector.tensor_copy(out=prefix_sb[:, tcch, k, :], in_=pfx[:])

        # second AllGather for the aux partial sums (overlaps scatter tail)
        nc.vector.tensor_copy(out=agin2[:, 0:E], in_=probsum_sb[:])
        nc.vector.tensor_copy(out=agin2[:, E:2 * E], in_=impsum_sb[:])
        ag2_in_d = dram.tile([1, 2 * E], fp32)
        ag2_out_d = dram.tile([NCORES, 2 * E], fp32, addr_space="Shared")
        nc.gpsimd.dma_start(out=ag2_in_d[:], in_=agin2[:])
        nc.gpsimd.collective_compute(
            "AllGather", mybir.AluOpType.bypass,
            ins=[ag2_in_d[:]], outs=[ag2_out_d[:]], replica_groups=RG,
        )
        nc.gpsimd.dma_start(out=aux_all[:], in_=ag2_out_d[:])

        # per-core totals per k (reduce the chunk axis), then offsets
        for k in range(2):
            nc.vector.tensor_reduce(
                out=tot_k[:, k, :],
                in_=cnt_all[:, k * NCH * E:(k + 1) * NCH * E]
                    .rearrange("p (n e) -> p e n", n=NCH),
                axis=AX.X, op=OP.add)
        ms0 = ccps_p.tile([1, E], fp32, tag="ccp")
        nc.tensor.matmul(out=ms0[:], lhsT=cmask_sb[:, :], rhs=tot_k[:, 0, :],
                         start=True, stop=True)
        nc.vector.tensor_copy(out=off0[:], in_=ms0[:])
        t0p = ccps_p.tile([1, E], fp32, tag="ccp")
        nc.tensor.matmul(out=t0p[:], lhsT=ones8, rhs=tot_k[:, 0, :],
                         start=True, stop=True)
        nc.vector.tensor_copy(out=tot0[:], in_=t0p[:])
        ms1 = ccps_p.tile([1, E], fp32, tag="ccp")
        nc.tensor.matmul(out=ms1[:], lhsT=cmask_sb[:, :], rhs=tot_k[:, 1, :],
                         start=True, stop=True)
        tt(off1[:], ms1[:], tot0[:], OP.add)

        for tcch in range(NCH):
            for k in range(2):
                sl = (tcch * 2 + k) * E
                offk = off0 if k == 0 else off1
                if tcch == 0:
                    nc.vector.tensor_copy(out=addv[:, sl:sl + E], in_=offk[:])
                else:
                    tt(addv[:, sl:sl + E], carr[:, k, tcch, :], offk[:], OP.add)
        bc_ps = l2ps.tile([P, NCH * 2 * E], fp32, tag="l2")
        nc.tensor.matmul(out=bc_ps[:], lhsT=ones_row, rhs=addv[:, :],
                         start=True, stop=True)

        # final positions + single-element scatter, batched over (tc,k):
        # flat index (t*E + e)*CAP + pos; dispatch writes 1.0, combine w.
        disp_flat = disp_d[:, :].rearrange("r c -> (r c)")[:, None]
        comb_flat = comb_d[:, :].rearrange("r c -> (r c)")[:, None]
        G = NCH * 2
        nc.vector.memset(ones_pk[:], 1.0)
        padd = sm.tile([P, G * E], fp32, tag="padd")
        tt(padd[:], prefix_sb[:, :, :, :], bc_ps[:, :], OP.add)
        scr = sm.tile([P, G * E], fp32, tag="scr")
        tt(scr[:], padd[:], ohs[:, :, :, :], OP.mult)
        posk = sm.tile([P, G], fp32, tag="posk")
        nc.vector.tensor_reduce(out=posk[:],
                                in_=scr[:].rearrange("p (g e) -> p g e", g=G),
                                axis=AX.X, op=OP.add)
        keep = sm.tile([P, G], fp32, tag="keep")
        ts(keep[:], posk[:], float(CAP), OP.is_lt)
        posc = sm.tile([P, G], fp32, tag="posc")
        ts(posc[:], posk[:], float(CAP - 1), OP.min)
        r0 = sm.tile([P, G], fp32, tag="r0")
        ts(r0[:], idx_sb[:, :, :], float(CAP), OP.mult)
        r1 = sm.tile([P, G], fp32, tag="r1")
        tt(r1[:], r0[:], tcoff, OP.add)
        r2 = sm.tile([P, G], fp32, tag="r2")
        ts(r2[:], r1[:], iota_tok, OP.add)
        r3 = sm.tile([P, G], fp32, tag="r3")
        tt(r3[:], r2[:], posc[:], OP.add)
        nk = sm.tile([P, G], fp32, tag="nk")
        ts(nk[:], keep[:], -BIG, OP.mult, BIG, OP.add)
        rf = sm.tile([P, G], fp32, tag="rf")
        tt(rf[:], r3[:], nk[:], OP.add)
        nc.vector.tensor_copy(out=ri_all[:], in_=rf[:])

        for g in range(G):
            tcch, k = divmod(g, 2)
            s1 = nc.gpsimd.indirect_dma_start(
                out=disp_flat, out_offset=IOA(ap=ri_all[:, g:g + 1], axis=0),
                in_=ones_col, in_offset=None,
                bounds_check=TPC * E * CAP - 1, oob_is_err=False)
            s2 = nc.gpsimd.indirect_dma_start(
                out=comb_flat, out_offset=IOA(ap=ri_all[:, g:g + 1], axis=0),
                in_=w_sb[:, tcch, k:k + 1], in_offset=None,
                bounds_check=TPC * E * CAP - 1, oob_is_err=False)
            for z in zfill_insts["dispatch"]:
                add_dep_helper(s1.ins, z.ins, reason="scatter after zero-fill")
            for z in zfill_insts["combine"]:
                add_dep_helper(s2.ins, z.ins, reason="scatter after zero-fill")

        # ---------------- aux loss (from the aux AllGather) ----------------
        def ln_series(out_sb, d_sb, tagp):
            # ln(1+d) = d*(1 - d*(1/2 - d*(1/3 - ...)))
            s = persist.tile([1, E], fp32, name=f"{tagp}_s")
            nc.vector.memset(s[:], 1.0 / NTERMS)
            for i in range(NTERMS - 1, 0, -1):
                mtmp = sm.tile([1, E], fp32, tag=f"{tagp}_m")
                tt(mtmp[:], d_sb[:], s[:], OP.mult)
                ts(s[:], mtmp[:], -1.0, OP.mult, 1.0 / i, OP.add)
            tt(out_sb, d_sb[:], s[:], OP.mult)

        rpp = ccps_p.tile([1, E], fp32, tag="ccp")
        nc.tensor.matmul(out=rpp[:], lhsT=ones8, rhs=aux_all[:, 0:E],
                         start=True, stop=True)
        ts(rppe_sb[:], rpp[:], 1.0 / N, OP.mult)
        imt = ccps_p.tile([1, E], fp32, tag="ccp")
        nc.tensor.matmul(out=imt[:], lhsT=ones8, rhs=aux_all[:, E:2 * E],
                         start=True, stop=True)
        ts(ims_sb[:], imt[:], 1.0e-9, OP.add)

        t8 = persist.tile([1, E], fp32)
        ts(t8[:], rppe_sb[:], 8.0, OP.mult, 1.0e-9, OP.add)
        d1 = persist.tile([1, E], fp32)
        ts(d1[:], t8[:], -1.0, OP.add)
        ln1 = persist.tile([1, E], fp32)
        ln_series(ln1[:], d1, "lnA")
        elt = persist.tile([1, E], fp32)
        tt(elt[:], rppe_sb[:], ln1[:], OP.mult)
        el = persist.tile([1, 1], fp32)
        nc.vector.tensor_reduce(out=el[:], in_=elt[:], axis=AX.X, op=OP.add)

        st = persist.tile([1, 1], fp32)
        nc.vector.tensor_reduce(out=st[:], in_=ims_sb[:], axis=AX.X, op=OP.add)
        rst0 = persist.tile([1, 1], fp32)
        nc.vector.reciprocal(out=rst0[:], in_=st[:])
        at1 = persist.tile([1, 1], fp32)
        tt(at1[:], st[:], rst0[:], OP.mult)
        at2 = persist.tile([1, 1], fp32)
        ts(at2[:], at1[:], -1.0, OP.mult, 2.0, OP.add)
        rst = persist.tile([1, 1], fp32)
        tt(rst[:], rst0[:], at2[:], OP.mult)
        ippe = persist.tile([1, E], fp32)
        ts(ippe[:], ims_sb[:], rst[:], OP.mult)
        u8 = persist.tile([1, E], fp32)
        ts(u8[:], ippe[:], 8.0, OP.mult, 8.0e-9, OP.add)
        d2 = persist.tile([1, E], fp32)
        ts(d2[:], u8[:], -1.0, OP.add)
        ln2 = persist.tile([1, E], fp32)
        ln_series(ln2[:], d2, "lnB")
        lnip = persist.tile([1, E], fp32)
        ts(lnip[:], ln2[:], -LN8, OP.add)
        iet = persist.tile([1, E], fp32)
        tt(iet[:], ippe[:], lnip[:], OP.mult)
        ies = persist.tile([1, 1], fp32)
        nc.vector.tensor_reduce(out=ies[:], in_=iet[:], axis=AX.X, op=OP.add)
        # aux = el + (0.1/ln8) * sum(ippe*ln(ippe+eps))   [ies = -imp_entropy]
        sc = persist.tile([1, 1], fp32)
        ts(sc[:], ies[:], 0.1 / LN8, OP.mult)
        auxv = persist.tile([1, 1], fp32)
        tt(auxv[:], el[:], sc[:], OP.add)
        nc.sync.dma_start(out=aux_d[:, :], in_=auxv[:])

    nc.compile()
    return nc




def _host_consts():
    if "cconst" not in _CACHE:
        cb = np.zeros((P, 274), np.float32)
        cb[:, 0:P] = np.eye(P, dtype=np.float32)
        cb[:, P:2 * P] = np.triu(np.ones((P, P), np.float32), 1)
        cb[:, 2 * P:2 * P + E] = np.tile(np.arange(E, dtype=np.float32), (P, 1))
        cb[:, 264] = np.arange(P, dtype=np.float32) * (E * CAP)
        cb[:, 265] = 1.0
        for j in range(8):
            cb[:, 266 + j] = (j // 2) * (P * E * CAP)
        _CACHE["cconst"] = cb
    return _CACHE["cconst"]

def _get_nc():
    if "nc" not in _CACHE:
        _CACHE["nc"] = _build()
    return _CACHE["nc"]


def kernel(**inputs):
    global LAST_EXEC_NS, LAST_TRACE_DIR
    from concourse.bass_utils import run_bass_kernel_spmd

    inp = {k: np.ascontiguousarray(np.asarray(v), dtype=np.float32)
           for k, v in inputs.items()}
    x = inp["hidden_states"].reshape(N, H)

    nc = _get_nc()
    in_maps = []
    for c in range(NCORES):
        in_maps.append(dict(
            hidden_states=np.ascontiguousarray(x[c * TPC:(c + 1) * TPC]),
            wi1=inp["wi1"], bi1=inp["bi1"], wi2=inp["wi2"],
            bi2=inp["bi2"].reshape(1, 1),
            wr1=inp["wr1"], br1=inp["br1"], wr2=inp["wr2"],
            br2=inp["br2"].reshape(1, E),
            wu1=inp["wu1"], bu1=inp["bu1"], wu2=inp["wu2"],
            bu2=inp["bu2"].reshape(1, E),
            cmask=(np.arange(NCORES) < c).astype(np.float32).reshape(NCORES, 1),
            cconst=_host_consts(),
            crow=np.ones((1, TPC), np.float32),
        ))

    trace = bool(int(os.environ.get("KERNEL_TRACE", "0")))
    kwargs = {}
    if trace:
        _install_ntff_hook()
        import tempfile
        LAST_TRACE_DIR = tempfile.mkdtemp(prefix="adaptive_router_trace_")
        kwargs["tmpdir"] = LAST_TRACE_DIR
    res = run_bass_kernel_spmd(nc, in_maps, core_ids=list(range(NCORES)),
                               trace=trace, **kwargs)
    LAST_EXEC_NS = res.exec_time_ns

    disp = np.concatenate(
        [res.results[c]["dispatch"].reshape(TPC, E, CAP) for c in range(NCORES)], 0
    ).reshape(B, S, E, CAP)
    comb = np.concatenate(
        [res.results[c]["combine"].reshape(TPC, E, CAP) for c in range(NCORES)], 0
    ).reshape(B, S, E, CAP)
    probs = np.concatenate(
        [res.results[c]["probs"] for c in range(NCORES)], 0).reshape(B, S, E)
    impv = np.concatenate(
        [res.results[c]["importance"] for c in range(NCORES)], 0).reshape(B, S, 1)
    aux = np.float32(res.results[0]["aux"].reshape(()))
    return disp, comb, probs, aux, impv
